# revision 36
# baseline (speedup 1.0000x reference)
"""MoE transformer block on 8 TRN2 NeuronCores.

Data-parallel over batch (4 batches = 784 tokens per core), no
collectives.  ~738 us HW exec (from a 1255 us fp32r baseline), max rel
err ~9e-3 vs the fp32 reference.

- Attention in bf16: Wq/Wk/Wv/Wo host-cast to bf16 (ACT HWDGE ring);
  q/k/v host-padded to 896 rows, cast bf16, laid out block-major
  [DK, 896, 128] so each xbar transpose-DMA (SP ring) reads one
  contiguous region; no PE transposes on the load path.
- Scores/softmax/ctx per (head, batch) at N=196 — bf16 matmuls have no
  min-free-dim constraint, so no batch-pair packing of the query dim.
  Both token halves' head chains interleave inside one loop so their
  serial softmax tails overlap.
- MoE FFNs in fp8e4m3 with DoubleRow matmuls (256-row contraction per
  pass, ~2x PE throughput): W1/W2 host-scaled by 256 and cast; x
  re-quantized to fp8 after LN1; h = gelu(psum/256 + b1) evicts
  straight to fp8; the 1/256 descale of the W2 product rides in the
  sel8 gate-broadcast selector.
- LayerNorm (stage-major across both token halves), softmax
  normalization, and gating stay in fp32/fp32r.

PSUM discipline: two pools, one unified tag each (every psum tile <= 1
bank, 4 bufs per pool -> exactly 8 banks).  The MoE y-phase holds 2+2
accumulators across the K(=F/2) loop while the next expert's h-phase
double-buffers 1+1.
"""
import sys

sys.path.insert(0, "/opt/trn_rl_repo")

from contextlib import ExitStack

import ml_dtypes
import numpy as np

import concourse.bass as bass
import concourse.tile as tile
from concourse import bacc, mybir
from concourse.bass_utils import run_bass_kernel_spmd
from concourse.masks import make_identity

FP32 = mybir.dt.float32
FP32R = mybir.dt.float32r
BF16 = mybir.dt.bfloat16
F8E4 = mybir.dt.float8e4
AF = mybir.ActivationFunctionType
OP = mybir.AluOpType
DR = mybir.MatmulPerfMode.DoubleRow

B, S, D, H, E, F = 32, 196, 768, 12, 8, 3072
DH = D // H                 # 64
NCORES = 8
BPC = B // NCORES           # 4 batches per core
T = BPC * S                 # 784 tokens per core
TP = 896                    # padded token count (multiple of 128) for xbar
TH = T // 2                 # 392 tokens per half (2 batches)
DK = D // 128               # 6
FK = F // 128               # 24
EPS = 1e-5
W8SCALE = 256.0             # host-side fp8 weight scale
BK = [(0, 128), (128, 68)]                              # ki chunks per batch
NCH = ((0, TH), (TH, TH))                               # token halves
# output token tiles, grouped by LN2 half
TTH = ([(0, 128), (128, 128), (256, 128), (384, 8)],
       [(392, 128), (520, 128), (648, 128), (776, 8)])

_CACHE = {}


def _build():
    nc = bacc.Bacc("TRN2", target_bir_lowering=False, debug=False,
                   num_devices=NCORES)

    q_d = nc.dram_tensor("q", [DK, TP, 128], BF16, kind="ExternalInput").ap()
    k_d = nc.dram_tensor("k", [DK, TP, 128], BF16, kind="ExternalInput").ap()
    v_d = nc.dram_tensor("v", [DK, TP, 128], BF16, kind="ExternalInput").ap()
    wq_d = nc.dram_tensor("Wq", [D, D], BF16, kind="ExternalInput").ap()
    wk_d = nc.dram_tensor("Wk", [D, D], BF16, kind="ExternalInput").ap()
    wv_d = nc.dram_tensor("Wv", [D, D], BF16, kind="ExternalInput").ap()
    wo_d = nc.dram_tensor("Wo", [D, D], BF16, kind="ExternalInput").ap()
    bq_d = nc.dram_tensor("bq", [D], FP32, kind="ExternalInput").ap()
    bk_d = nc.dram_tensor("bk", [D], FP32, kind="ExternalInput").ap()
    bv_d = nc.dram_tensor("bv", [D], FP32, kind="ExternalInput").ap()
    bo_d = nc.dram_tensor("bo", [D], FP32, kind="ExternalInput").ap()
    l1g_d = nc.dram_tensor("ln1_g", [D], FP32, kind="ExternalInput").ap()
    l1b_d = nc.dram_tensor("ln1_b", [D], FP32, kind="ExternalInput").ap()
    l2g_d = nc.dram_tensor("ln2_g", [D], FP32, kind="ExternalInput").ap()
    l2b_d = nc.dram_tensor("ln2_b", [D], FP32, kind="ExternalInput").ap()
    wg_d = nc.dram_tensor("Wg", [D, E], FP32, kind="ExternalInput").ap()
    bg_d = nc.dram_tensor("bg", [E], FP32, kind="ExternalInput").ap()
    w1_d = nc.dram_tensor("W1f8", [E, D, F], F8E4, kind="ExternalInput").ap()
    b1_d = nc.dram_tensor("b1", [E, F], FP32, kind="ExternalInput").ap()
    w2_d = nc.dram_tensor("W2f8", [E, F, D], F8E4, kind="ExternalInput").ap()
    b2_d = nc.dram_tensor("b2", [E, D], FP32, kind="ExternalInput").ap()
    sel_d = nc.dram_tensor("sel8", [E, E * 128], FP32,
                           kind="ExternalInput").ap()
    aux1_d = nc.dram_tensor("aux_ones", [128, 128], FP32,
                            kind="ExternalInput").ap()
    auxb_d = nc.dram_tensor("aux_ones_bf", [128, 128], BF16,
                            kind="ExternalInput").ap()
    aux64_d = nc.dram_tensor("aux_ones64", [65, 128], FP32,
                             kind="ExternalInput").ap()
    auxe_d = nc.dram_tensor("aux_eps", [1, 1], FP32,
                            kind="ExternalInput").ap()
    out_d = nc.dram_tensor("out", [T, D], FP32, kind="ExternalOutput").ap()

    with tile.TileContext(nc) as tc, ExitStack() as top:
        const = top.enter_context(tc.tile_pool(name="const", bufs=1))
        vecs = top.enter_context(tc.tile_pool(name="vecs", bufs=1))
        rows = top.enter_context(tc.tile_pool(name="rows", bufs=2))
        psA = top.enter_context(tc.tile_pool(name="psA", bufs=4, space="PSUM"))
        psB = top.enter_context(tc.tile_pool(name="psB", bufs=4, space="PSUM"))
        tmp = top.enter_context(tc.tile_pool(name="tmp", bufs=2))
        persist = top.enter_context(tc.tile_pool(name="persist", bufs=1))

        def pa(p, f):
            return psA.tile([p, f], FP32, tag="a", name="pa")

        def pb(p, f):
            return psB.tile([p, f], FP32, tag="b", name="pb")

        # ---------------- constants ----------------
        ident = const.tile([128, 128], FP32, tag="ident")
        make_identity(nc, ident)
        ones_col_r = const.tile([128, 1], FP32R, tag="ones_col_r")
        nc.gpsimd.dma_start(out=ones_col_r[:], in_=aux1_d[:, 0:1])
        ones_row_r = const.tile([1, 128], FP32R, tag="ones_row_r")
        nc.gpsimd.dma_start(out=ones_row_r[:], in_=aux1_d[0:1, :])
        ones_row8_r = const.tile([1, 8], FP32R, tag="ones_row8_r")
        nc.gpsimd.dma_start(out=ones_row8_r[:], in_=aux1_d[0:1, 0:8])
        ones8_col = const.tile([8, 1], FP32, tag="ones8_col")
        nc.gpsimd.dma_start(out=ones8_col[:], in_=aux1_d[0:8, 0:1])
        ones_col_b = const.tile([128, 1], BF16, tag="ones_col_b")
        nc.gpsimd.dma_start(out=ones_col_b[:], in_=auxb_d[:, 0:1])
        # row 64 all-ones: lhsT for the 1/s broadcast (base matches pctx[64])
        ones64r = const.tile([65, 128], FP32R, tag="ones64r")
        nc.gpsimd.dma_start(out=ones64r[:], in_=aux64_d[:, :])
        eps_t = const.tile([1, 1], FP32, tag="eps")
        nc.gpsimd.dma_start(out=eps_t[:], in_=auxe_d[:, :])
        # per-expert selector: sel8[i, e*128+p] = (i==e)/256 (fp8 descale)
        sel8 = const.tile([8, E * 128], FP32R, tag="sel8")
        nc.gpsimd.dma_start(out=sel8[:], in_=sel_d[:, :])

        def load_col(dvec, nb, dtype=FP32, tag=None):
            # [nb*128] DRAM vector -> [128, nb] feature-major column tile
            # (SWDGE ring: keeps the HWDGE rings clear for the big loads)
            raw = rows.tile([nb, 128], FP32, tag="rawvec")
            nc.scalar.dma_start(out=raw[:],
                                in_=dvec.rearrange("(a b) -> a b", b=128))
            ps = pb(128, nb)
            nc.tensor.transpose(ps[:], raw[:], ident[:nb, :nb])
            col = vecs.tile([128, nb], dtype, tag=tag)
            nc.vector.tensor_copy(col[:], ps[:])
            return col

        wgs = vecs.tile([128, DK, E], FP32R, tag="wg")
        nc.gpsimd.dma_start(
            out=wgs[:], in_=wg_d.rearrange("(kb p) e -> p kb e", p=128))
        b2s = vecs.tile([E, D], FP32R, tag="b2")
        nc.gpsimd.dma_start(out=b2s[:], in_=b2_d[:, :])

        # persistent activations (full T)
        x_t = [persist.tile([128, T], FP32R, tag=f"xt{k}", name=f"xt{k}")
               for k in range(DK)]
        x8 = [persist.tile([128, 2, T], F8E4, tag=f"x8_{dp}", name=f"x8_{dp}")
              for dp in range(3)]
        moe = [persist.tile([128, T], FP32, tag=f"moe{k}", name=f"moe{k}")
               for k in range(DK)]
        bias_total = vecs.tile([128, DK], FP32, tag="btot")

        def layer_norm(jobs):
            # feature-major LN over D=768 partitions (6 tiles), fp32r input.
            # jobs: list of (r_tiles, g_col, b_col, out_tiles, in_off,
            # out_off, nl, ones_c) — stage-major across jobs so one job's
            # serial stats chain overlaps the other's matmuls/evictions.
            # ones_c matches r_tiles' dtype (PE operand pairing).
            st = []
            for (r_tiles, g_col, b_col, out_tiles, n0, o0, nl, oc) in jobs:
                ps_s = pa(1, TH)
                ps_s2 = pa(1, TH)
                for k in range(DK):
                    sq = tmp.tile([128, TH], FP32R, tag="ln_sq", bufs=3)
                    nc.scalar.activation(sq[:], r_tiles[k][:, n0:n0 + nl],
                                         AF.Square)
                    nc.tensor.matmul(ps_s[:], oc[:],
                                     r_tiles[k][:, n0:n0 + nl],
                                     start=(k == 0), stop=(k == DK - 1))
                    nc.tensor.matmul(ps_s2[:], ones_col_r[:], sq[:],
                                     start=(k == 0), stop=(k == DK - 1))
                st.append((ps_s, ps_s2))
            br = []
            for ji, (r_tiles, g_col, b_col, out_tiles, n0, o0, nl, oc) in \
                    enumerate(jobs):
                ps_s, ps_s2 = st[ji]
                m = rows.tile([1, TH], FP32, tag="ln_m", bufs=2)
                m2 = rows.tile([1, TH], FP32, tag="ln_m2", bufs=2)
                nc.vector.tensor_scalar_mul(m[:], ps_s[:], 1.0 / D)
                nc.vector.tensor_scalar_mul(m2[:], ps_s2[:], 1.0 / D)
                mm_ = rows.tile([1, TH], FP32, tag="ln_mm", bufs=2)
                nc.vector.tensor_mul(mm_[:], m[:], m[:])
                var = rows.tile([1, TH], FP32, tag="ln_var", bufs=2)
                nc.vector.tensor_sub(var[:], m2[:], mm_[:])
                sd = rows.tile([1, TH], FP32, tag="ln_sd", bufs=2)
                nc.scalar.activation(sd[:], var[:], AF.Sqrt, bias=eps_t[:])
                rstd = rows.tile([1, TH], FP32R, tag="ln_rstd", bufs=2)
                with nc.allow_low_precision(reason="fp32r matmul operand"):
                    nc.vector.reciprocal(rstd[:], sd[:])
                mr = rows.tile([1, TH], FP32R, tag="ln_mr", bufs=2)
                nc.vector.tensor_mul(mr[:], m[:], rstd[:])
                pR = pb(128, TH)
                nc.tensor.matmul(pR[:], ones_row_r[:], rstd[:],
                                 start=True, stop=True)
                pM = pb(128, TH)
                nc.tensor.matmul(pM[:], ones_row_r[:], mr[:],
                                 start=True, stop=True)
                br.append((pR, pM))
            for ji, (r_tiles, g_col, b_col, out_tiles, n0, o0, nl, oc) in \
                    enumerate(jobs):
                pR, pM = br[ji]
                for k in range(DK):
                    t1 = tmp.tile([128, TH], FP32, tag="ln_t1")
                    nc.vector.tensor_mul(t1[:], r_tiles[k][:, n0:n0 + nl],
                                         pR[:])
                    t2 = tmp.tile([128, TH], FP32, tag="ln_t2")
                    nc.vector.tensor_sub(t2[:], t1[:], pM[:])
                    nc.scalar.activation(out_tiles[k][:, o0:o0 + nl],
                                         t2[:], AF.Identity,
                                         bias=b_col[:, k:k + 1],
                                         scale=g_col[:, k:k + 1])

        # ================= attention =================
        with ExitStack() as hs:
            paw = hs.enter_context(tc.tile_pool(name="paw", bufs=3))
            pq = hs.enter_context(tc.tile_pool(name="pq", bufs=1))
            pproj = hs.enter_context(tc.tile_pool(name="pproj", bufs=1))
            phv = hs.enter_context(tc.tile_pool(name="phv", bufs=8))
            kvs = ExitStack()
            pkv = kvs.enter_context(tc.tile_pool(name="pkv", bufs=6))

            def load_w(dram):
                # 3 slots: wq/wk/wv stream back-to-back; wo reuses wq's
                wt = paw.tile([128, DK, D], BF16, tag="w", name="w")
                nc.scalar.dma_start(
                    out=wt[:], in_=dram.rearrange("(kb p) d -> p kb d", p=128))
                return wt

            # ---- q feature-major via xbar transpose-DMA, then qh ----
            wq = load_w(wq_d)
            q_t = [pq.tile([128, TP], BF16, tag=f"qt{k}", name=f"qt{k}")
                   for k in range(DK)]
            for k in range(DK):
                nc.sync.dma_start(out=q_t[k][:], in_=q_d[k],
                                  transpose=True)
            # column-vector loads ride the scalar ring behind wq, ready
            # before the first eviction needs them
            bq_col = load_col(bq_d, DK, tag="bq")
            bk_col = load_col(bk_d, DK, tag="bk")
            bo_col = load_col(bo_d, DK, tag="bo")
            bv_col = load_col(bv_d, DK, BF16, tag="bv")
            l1g_col = load_col(l1g_d, DK, tag="l1g")
            l1b_col = load_col(l1b_d, DK, tag="l1b")
            l2g_col = load_col(l2g_d, DK, tag="l2g")
            l2b_col = load_col(l2b_d, DK, tag="l2b")
            bg_col = vecs.tile([8, 1], FP32, tag="bg")
            nc.scalar.dma_start(out=bg_col[:],
                                in_=bg_d.rearrange("(a b) -> a b", b=1))

            qh_t = [pproj.tile([128, T], BF16, tag=f"qh{k}", name=f"qh{k}")
                    for k in range(DK)]
            kh_t = [pproj.tile([128, T], BF16, tag=f"kh{k}", name=f"kh{k}")
                    for k in range(DK)]

            def project(w, src, dst, bcol, scope):
                with nc.named_scope(scope):
                    for (n0, nl) in NCH:
                        for mi in range(DK):
                            ps = pa(128, TH)
                            for k in range(DK):
                                nc.tensor.matmul(
                                    ps[:], w[:, k, mi * 128:(mi + 1) * 128],
                                    src[k][:, n0:n0 + nl],
                                    start=(k == 0), stop=(k == DK - 1))
                            nc.scalar.activation(dst[mi][:, n0:n0 + nl],
                                                 ps[:], AF.Identity,
                                                 bias=bcol[:, mi:mi + 1])

            project(wq, q_t, qh_t, bq_col, "proj_q")

            # ---- k, kh ----
            wk = load_w(wk_d)
            k_t = [pkv.tile([128, TP], BF16, tag="kv", name=f"kt{k}")
                   for k in range(DK)]
            for k in range(DK):
                nc.sync.dma_start(out=k_t[k][:], in_=k_d[k],
                                  transpose=True)
            project(wk, k_t, kh_t, bk_col, "proj_k")

            # ---- v, vh (all 4 batches), token-major, ones col ----
            wv = load_w(wv_d)
            v_t = [pkv.tile([128, TP], BF16, tag="kv", name=f"vt{k}")
                   for k in range(DK)]
            for k in range(DK):
                nc.sync.dma_start(out=v_t[k][:], in_=v_d[k],
                                  transpose=True)
            vh = {}
            with nc.named_scope("vh"):
                for bt in range(4):
                    for ci, (c0, cl) in enumerate(BK):
                        vt_ = phv.tile([128, H, DH + 1], BF16,
                                       tag="vh", name=f"vh{bt}{ci}")
                        nc.scalar.dma_start(out=vt_[:cl, :, DH:DH + 1],
                                            in_=auxb_d[:cl, 0:H])
                        tc0 = bt * S + c0
                        for ni in range(2):
                            ps = pa(128, 384)
                            for k in range(DK):
                                nc.tensor.matmul(
                                    ps[:cl, :], v_t[k][:, tc0:tc0 + cl],
                                    wv[:, k, ni * 384:(ni + 1) * 384],
                                    start=(k == 0), stop=(k == DK - 1))
                            nc.vector.tensor_copy(
                                vt_[:cl, ni * 6:(ni + 1) * 6, 0:DH],
                                ps[:cl, :].rearrange("p (h d) -> p h d",
                                                     d=DH))
                        vh[(bt, ci)] = vt_

            # k_t/v_t dead past here — release their SBUF before the
            # head-loop pools open
            kvs.close()

            # ---- Wo + bias_total = Wo^T bv + bo ----
            wo = load_w(wo_d)
            for mi in range(DK):
                pbs = pb(128, 1)
                for k in range(DK):
                    nc.tensor.matmul(pbs[:],
                                     wo[:, k, mi * 128:(mi + 1) * 128],
                                     bv_col[:, k:k + 1],
                                     start=(k == 0), stop=(k == DK - 1))
                nc.vector.tensor_add(bias_total[:, mi:mi + 1], pbs[:],
                                     bo_col[:, mi:mi + 1])

            with ExitStack() as ph_:
                phe = ph_.enter_context(tc.tile_pool(name="phe", bufs=4))
                pho = ph_.enter_context(tc.tile_pool(name="pho", bufs=3))
                pcx = ph_.enter_context(tc.tile_pool(name="pcx", bufs=1))

                # ---- attention, batch-pair packed (N=392, bf16) ----
                # Both halves' head chains interleave: while one half's
                # softmax tail drains, the other half's scores/ctx matmuls
                # keep the PE fed.
                cxp2 = [[pcx.tile([128, TH], BF16, tag=f"cx{half}{mi}",
                                  name=f"cx{half}{mi}") for mi in range(DK)]
                        for half in range(2)]
                with nc.named_scope("heads"):
                    for hh in range(H):
                        dm, ro = divmod(hh * DH, 128)
                        for half in range(2):
                            h0tok = half * TH
                            cxp = cxp2[half]
                            pctxs = []
                            for bl in range(2):
                                bq0 = h0tok + bl * S
                                # both ki-chunks' scores^T share one psum
                                # bank (196-col slices at 256 offsets) so
                                # ONE exp call covers them
                                ps = psA.tile([128, 2, 256], FP32, tag="a",
                                              name="psc")
                                for ci, (c0, cl) in enumerate(BK):
                                    nc.tensor.matmul(
                                        ps[:cl, ci, 0:S],
                                        kh_t[dm][ro:ro + DH,
                                                 bq0 + c0:bq0 + c0 + cl],
                                        qh_t[dm][ro:ro + DH, bq0:bq0 + S],
                                        start=True, stop=True)
                                ex = phe.tile([128, 2, S], BF16, tag="exp",
                                              bufs=6)
                                nc.scalar.activation(ex[:], ps[:, :, 0:S],
                                                     AF.Exp, scale=0.125)
                                pctx = pb(DH + 1, S)
                                for ci, (c0, cl) in enumerate(BK):
                                    nc.tensor.matmul(
                                        pctx[:],
                                        vh[(half * 2 + bl, ci)][:cl, hh, :],
                                        ex[:cl, ci, :],
                                        start=(ci == 0), stop=(ci == 1))
                                pctxs.append(pctx)
                            srec = rows.tile([65, TH], FP32R, tag="srec",
                                             bufs=4)
                            with nc.allow_low_precision(reason="fp32r rep"):
                                nc.vector.reciprocal(srec[64:65, 0:S],
                                                     pctxs[0][64:65, 0:S])
                                nc.vector.reciprocal(srec[64:65, S:TH],
                                                     pctxs[1][64:65, 0:S])
                            prep = pb(DH, TH)
                            nc.tensor.matmul(prep[:], ones64r[64:65, 0:DH],
                                             srec[64:65, :],
                                             start=True, stop=True)
                            prs = phe.tile([64, TH], BF16, tag="prs", bufs=3)
                            nc.vector.tensor_copy(prs[:], prep[:])
                            for bl in range(2):
                                bc = bl * S
                                if ro == 0:
                                    nc.vector.tensor_mul(
                                        cxp[dm][0:DH, bc:bc + S],
                                        pctxs[bl][0:DH, 0:S],
                                        prs[:, bc:bc + S])
                                else:
                                    co = pho.tile([64, S], BF16, tag="cxodd")
                                    nc.vector.tensor_mul(
                                        co[:], pctxs[bl][0:DH, 0:S],
                                        prs[:, bc:bc + S])
                                    nc.scalar.dma_start(
                                        out=cxp[dm][64:128, bc:bc + S],
                                        in_=co[:])

                # all experts' b1, feature-major [128, E, FK] — PE work here
                # fills the LN1 stats-chain bubbles
                b1c = vecs.tile([128, E, FK], FP32, tag="b1c")
                for e in range(E):
                    braw = rows.tile([FK, 128], FP32, tag="rawb1")
                    nc.sync.dma_start(
                        out=braw[:],
                        in_=b1_d[e].rearrange("(a b) -> a b", b=128))
                    pbv = pb(128, FK)
                    nc.tensor.transpose(pbv[:], braw[:], ident[:FK, :FK])
                    nc.vector.tensor_copy(b1c[:, e, :], pbv[:])

                # ---- Wo projection + residual -> r1, LN1 -> x, x8 ----
                with nc.named_scope("wo_ln"):
                    r1h = [[pcx.tile([128, TH], BF16, tag=f"r1{half}{mi}",
                                     name=f"r1{half}{mi}")
                            for mi in range(DK)] for half in range(2)]
                    for mi in range(DK):
                        for half in range(2):
                            h0tok = half * TH
                            ps = pa(128, TH)
                            for k in range(DK):
                                nc.tensor.matmul(
                                    ps[:], wo[:, k, mi * 128:(mi + 1) * 128],
                                    cxp2[half][k][:],
                                    start=(k == 0), stop=(k == DK - 1))
                            nc.vector.scalar_tensor_tensor(
                                out=r1h[half][mi][:], in0=ps[:],
                                scalar=bias_total[:, mi:mi + 1],
                                in1=q_t[mi][:, h0tok:h0tok + TH],
                                op0=OP.add, op1=OP.add)

                    layer_norm(
                        [(r1h[0], l1g_col, l1b_col, x_t, 0, 0, TH,
                          ones_col_b),
                         (r1h[1], l1g_col, l1b_col, x_t, 0, TH, TH,
                          ones_col_b)])
                    for half in range(2):
                        h0tok = half * TH
                        for dp in range(3):
                            for i in range(2):
                                nc.vector.tensor_copy(
                                    x8[dp][:, i, h0tok:h0tok + TH],
                                    x_t[2 * dp + i][:, h0tok:h0tok + TH])

        # ================= gates =================
        gexp = persist.tile([8, T], FP32, tag="gexp")
        gate = persist.tile([8, T], FP32R, tag="gate")
        with nc.named_scope("gates"):
            # stage-major over the two token halves: one half's serial
            # softmax chain overlaps the other's matmuls
            pgl = []
            for (n0, nl) in NCH:
                pg = pb(8, TH)
                for k in range(DK):
                    nc.tensor.matmul(pg[:], wgs[:, k, :],
                                     x_t[k][:, n0:n0 + nl],
                                     start=(k == 0), stop=(k == DK - 1))
                nc.scalar.activation(gexp[:, n0:n0 + nl], pg[:], AF.Exp,
                                     bias=bg_col[:])
                pgl.append(pg)
            for ci, (n0, nl) in enumerate(NCH):
                pgs = pa(1, TH)
                nc.tensor.matmul(pgs[:], ones8_col[:], gexp[:, n0:n0 + nl],
                                 start=True, stop=True)
                grec = rows.tile([1, TH], FP32R, tag="grec", bufs=2)
                with nc.allow_low_precision(reason="fp32r matmul operand"):
                    nc.vector.reciprocal(grec[:], pgs[:])
                pgr = pb(8, TH)
                nc.tensor.matmul(pgr[:], ones_row8_r[:], grec[:],
                                 start=True, stop=True)
                nc.vector.tensor_mul(gate[:, n0:n0 + nl],
                                     gexp[:, n0:n0 + nl], pgr[:])

            # moe_acc init = gates^T @ b2   (lhsT = b2 chunks [8, 128])
            for mi in range(DK):
                for (n0, nl) in NCH:
                    pbi = pa(128, TH)
                    nc.tensor.matmul(pbi[:], b2s[:, mi * 128:(mi + 1) * 128],
                                     gate[:, n0:n0 + nl],
                                     start=True, stop=True)
                    nc.scalar.copy(moe[mi][:, n0:n0 + nl], pbi[:])

        # ================= MoE experts (fp8 DoubleRow) =================
        with ExitStack() as ms:
            pmh = ms.enter_context(tc.tile_pool(name="pmh", bufs=26))
            pmw1 = ms.enter_context(tc.tile_pool(name="pmw1", bufs=4))
            pmw2 = ms.enter_context(tc.tile_pool(name="pmw2", bufs=16))
            for e in range(E):
              with nc.named_scope(f"moe{e}"):
                # gate row broadcast to 128 partitions (carries 1/256 descale)
                grep = tmp.tile([128, T], BF16, tag="gerep")
                for (n0, nl) in NCH:
                    pge = pb(128, TH)
                    nc.tensor.matmul(pge[:],
                                     sel8[:, e * 128:(e + 1) * 128],
                                     gate[:, n0:n0 + nl],
                                     start=True, stop=True)
                    nc.vector.tensor_copy(grep[:, n0:n0 + nl], pge[:])

                # expert weights, fp8, DoubleRow pair layout
                w1t = []
                for dp in range(3):
                    wt = pmw1.tile([128, 2, F], F8E4, tag="w1", name="w1t")
                    nc.sync.dma_start(
                        out=wt[:],
                        in_=w1_d[e, dp * 256:(dp + 1) * 256, :].rearrange(
                            "(i p) f -> p i f", p=128))
                    w1t.append(wt)
                w2t = []
                for fbp in range(FK // 2):
                    wt = pmw2.tile([128, 2, D], F8E4, tag="w2", name="w2t")
                    nc.sync.dma_start(
                        out=wt[:],
                        in_=w2_d[e, fbp * 256:(fbp + 1) * 256, :].rearrange(
                            "(i p) d -> p i d", p=128))
                    w2t.append(wt)

                # ---- h = gelu((W1*256)^T x / 256 + b1) -> fp8 [F, T] ----
                hts = []
                for fm in range(FK):
                    fbp, ih = divmod(fm, 2)
                    if ih == 0:
                        hp = pmh.tile([128, 2, T], F8E4, tag="h", name="hp")
                        hts.append(hp)
                    ph0 = pa(128, TH)
                    ph1 = pb(128, TH)
                    for dp in range(3):
                        nc.tensor.matmul(
                            ph0[:], w1t[dp][:, :, fm * 128:(fm + 1) * 128],
                            x8[dp][:, :, 0:TH],
                            start=(dp == 0), stop=(dp == 2), perf_mode=DR)
                        nc.tensor.matmul(
                            ph1[:], w1t[dp][:, :, fm * 128:(fm + 1) * 128],
                            x8[dp][:, :, TH:T],
                            start=(dp == 0), stop=(dp == 2), perf_mode=DR)
                    nc.scalar.activation(hts[fbp][:, ih, 0:TH], ph0[:],
                                         AF.Gelu, bias=b1c[:, e, fm:fm + 1],
                                         scale=1.0 / W8SCALE)
                    nc.scalar.activation(hts[fbp][:, ih, TH:T], ph1[:],
                                         AF.Gelu, bias=b1c[:, e, fm:fm + 1],
                                         scale=1.0 / W8SCALE)

                # ---- y = (W2*256)^T h (K-accum in PSUM), combine ----
                for dg in range(3):
                    pys = [pa(128, TH) for _ in range(2)] + \
                          [pb(128, TH) for _ in range(2)]
                    for fbp in range(FK // 2):
                        for j in range(2):
                            m0 = dg * 256 + j * 128
                            for ni, (n0, nl) in enumerate(NCH):
                                nc.tensor.matmul(
                                    pys[j * 2 + ni][:],
                                    w2t[fbp][:, :, m0:m0 + 128],
                                    hts[fbp][:, :, n0:n0 + nl],
                                    start=(fbp == 0), stop=(fbp == 11),
                                    perf_mode=DR)
                    for j in range(2):
                        mi = dg * 2 + j
                        for ni, (n0, nl) in enumerate(NCH):
                            ty = tmp.tile([128, TH], FP32, tag="ty")
                            nc.vector.tensor_mul(ty[:], pys[j * 2 + ni][:],
                                                 grep[:, n0:n0 + nl])
                            nc.vector.tensor_add(moe[mi][:, n0:n0 + nl],
                                                 moe[mi][:, n0:n0 + nl],
                                                 ty[:])

        # ================= LN2 + output =================
        with ExitStack() as fs:
            pfo = fs.enter_context(tc.tile_pool(name="pfo", bufs=3))
            with nc.named_scope("tail"):
                for (n0, nl) in NCH:
                    for mi in range(DK):
                        nc.vector.tensor_add(x_t[mi][:, n0:n0 + nl],
                                             x_t[mi][:, n0:n0 + nl],
                                             moe[mi][:, n0:n0 + nl])
                layer_norm(
                    [(x_t, l2g_col, l2b_col, moe, n0, n0, nl, ones_col_r)
                     for (n0, nl) in NCH])
                for ci in range(2):
                    for (t0, tl) in TTH[ci]:
                        ot = pfo.tile([128, D], FP32, tag="otok")
                        for k in range(DK):
                            ps = pa(128, 128)
                            nc.tensor.transpose(ps[:tl, :],
                                                moe[k][:, t0:t0 + tl],
                                                ident[:, :])
                            nc.vector.tensor_copy(
                                ot[:tl, k * 128:(k + 1) * 128], ps[:tl, :])
                        oeng = nc.sync if (t0 // 128) % 2 == 0 else nc.scalar
                        oeng.dma_start(out=out_d[t0:t0 + tl, :],
                                       in_=ot[:tl, :])

    nc.compile()
    return nc


def _get_nc():
    if "nc" not in _CACHE:
        _CACHE["nc"] = _build()
    return _CACHE["nc"]


def run(inputs, **spmd_kwargs):
    nc = _get_nc()
    f32 = np.float32
    bf16 = ml_dtypes.bfloat16
    f8 = ml_dtypes.float8_e4m3
    inp = {k: np.asarray(v) for k, v in inputs.items()}
    shared = {}
    for name in ("bq", "bk", "bv", "bo", "ln1_g", "ln1_b", "ln2_g", "ln2_b",
                 "Wg", "bg", "b1", "b2"):
        shared[name] = np.ascontiguousarray(inp[name].astype(f32))
    for name in ("Wq", "Wk", "Wv", "Wo"):
        shared[name] = np.ascontiguousarray(inp[name].astype(f32).astype(bf16))
    shared["W1f8"] = np.ascontiguousarray(
        (inp["W1"].astype(f32) * W8SCALE).astype(f8))
    shared["W2f8"] = np.ascontiguousarray(
        (inp["W2"].astype(f32) * W8SCALE).astype(f8))
    sel = np.zeros((E, E * 128), dtype=f32)
    for e in range(E):
        sel[e, e * 128:(e + 1) * 128] = 1.0 / W8SCALE
    shared["sel8"] = sel
    shared["aux_ones"] = np.ones((128, 128), dtype=f32)
    shared["aux_ones_bf"] = np.ones((128, 128), dtype=bf16)
    a64 = np.zeros((65, 128), dtype=f32)
    a64[64, :] = 1.0
    shared["aux_ones64"] = a64
    shared["aux_eps"] = np.full((1, 1), EPS, dtype=f32)
    in_maps = []
    for c in range(NCORES):
        m = dict(shared)
        for name in ("q", "k", "v"):
            pad = np.zeros((TP, D), dtype=bf16)
            pad[:T] = inp[name][c * BPC:(c + 1) * BPC].reshape(
                T, D).astype(f32).astype(bf16)
            # block-major [DK, TP, 128]: each xbar transpose-DMA reads one
            # fully contiguous region
            m[name] = np.ascontiguousarray(
                pad.reshape(TP, DK, 128).transpose(1, 0, 2))
        in_maps.append(m)
    res = run_bass_kernel_spmd(nc, in_maps, core_ids=list(range(NCORES)),
                               **spmd_kwargs)
    out = np.stack([r["out"] for r in res.results])  # [8, T, D]
    return out.reshape(B, S, D), res


def kernel(**inputs):
    out, _ = run(inputs)
    return out


# revision 38
# speedup vs baseline: 1.0523x; 1.0523x over previous
"""MoE transformer block on 8 TRN2 NeuronCores.

Data-parallel over batch (4 batches = 784 tokens per core), no
collectives.  ~738 us HW exec (from a 1255 us fp32r baseline), max rel
err ~9e-3 vs the fp32 reference.

- Attention in bf16: Wq/Wk/Wv/Wo host-cast to bf16 (ACT HWDGE ring);
  q/k/v host-padded to 896 rows, cast bf16, laid out block-major
  [DK, 896, 128] so each xbar transpose-DMA (SP ring) reads one
  contiguous region; no PE transposes on the load path.
- Scores/softmax/ctx per (head, batch) at N=196 — bf16 matmuls have no
  min-free-dim constraint, so no batch-pair packing of the query dim.
  Both token halves' head chains interleave inside one loop so their
  serial softmax tails overlap.
- MoE FFNs in fp8e4m3 with DoubleRow matmuls (256-row contraction per
  pass, ~2x PE throughput): W1/W2 host-scaled by 256 and cast; x
  re-quantized to fp8 after LN1; h = gelu(psum/256 + b1) evicts
  straight to fp8; the 1/256 descale of the W2 product rides in the
  sel8 gate-broadcast selector.
- LayerNorm (stage-major across both token halves), softmax
  normalization, and gating stay in fp32/fp32r.

PSUM discipline: two pools, one unified tag each (every psum tile <= 1
bank, 4 bufs per pool -> exactly 8 banks).  The MoE y-phase holds 2+2
accumulators across the K(=F/2) loop while the next expert's h-phase
double-buffers 1+1.
"""
import sys

sys.path.insert(0, "/opt/trn_rl_repo")

from contextlib import ExitStack

import ml_dtypes
import numpy as np

import concourse.bass as bass
import concourse.tile as tile
from concourse import bacc, mybir
from concourse.bass_utils import run_bass_kernel_spmd
from concourse.masks import make_identity

FP32 = mybir.dt.float32
FP32R = mybir.dt.float32r
BF16 = mybir.dt.bfloat16
F8E4 = mybir.dt.float8e4
AF = mybir.ActivationFunctionType
OP = mybir.AluOpType
DR = mybir.MatmulPerfMode.DoubleRow

B, S, D, H, E, F = 32, 196, 768, 12, 8, 3072
DH = D // H                 # 64
NCORES = 8
BPC = B // NCORES           # 4 batches per core
T = BPC * S                 # 784 tokens per core
TP = 896                    # padded token count (multiple of 128) for xbar
TH = T // 2                 # 392 tokens per half (2 batches)
DK = D // 128               # 6
FK = F // 128               # 24
EPS = 1e-5
W8SCALE = 256.0             # host-side fp8 weight scale
BK = [(0, 128), (128, 68)]                              # ki chunks per batch
NCH = ((0, TH), (TH, TH))                               # token halves
# output token tiles, grouped by LN2 half
TTH = ([(0, 128), (128, 128), (256, 128), (384, 8)],
       [(392, 128), (520, 128), (648, 128), (776, 8)])

_CACHE = {}


def _build():
    nc = bacc.Bacc("TRN2", target_bir_lowering=False, debug=False,
                   num_devices=NCORES)

    q_d = nc.dram_tensor("q", [DK, TP, 128], BF16, kind="ExternalInput").ap()
    k_d = nc.dram_tensor("k", [DK, TP, 128], BF16, kind="ExternalInput").ap()
    v_d = nc.dram_tensor("v", [DK, TP, 128], BF16, kind="ExternalInput").ap()
    wq_d = nc.dram_tensor("Wq", [D, D], BF16, kind="ExternalInput").ap()
    wk_d = nc.dram_tensor("Wk", [D, D], BF16, kind="ExternalInput").ap()
    wv_d = nc.dram_tensor("Wv", [D, D], BF16, kind="ExternalInput").ap()
    wo_d = nc.dram_tensor("Wo", [D, D], BF16, kind="ExternalInput").ap()
    bq_d = nc.dram_tensor("bq", [D], FP32, kind="ExternalInput").ap()
    bk_d = nc.dram_tensor("bk", [D], FP32, kind="ExternalInput").ap()
    bv_d = nc.dram_tensor("bv", [D], FP32, kind="ExternalInput").ap()
    bo_d = nc.dram_tensor("bo", [D], FP32, kind="ExternalInput").ap()
    l1g_d = nc.dram_tensor("ln1_g", [D], FP32, kind="ExternalInput").ap()
    l1b_d = nc.dram_tensor("ln1_b", [D], FP32, kind="ExternalInput").ap()
    l2g_d = nc.dram_tensor("ln2_g", [D], FP32, kind="ExternalInput").ap()
    l2b_d = nc.dram_tensor("ln2_b", [D], FP32, kind="ExternalInput").ap()
    wg_d = nc.dram_tensor("Wg", [D, E], FP32, kind="ExternalInput").ap()
    bg_d = nc.dram_tensor("bg", [E], FP32, kind="ExternalInput").ap()
    w1_d = nc.dram_tensor("W1f8", [E, D, F], F8E4, kind="ExternalInput").ap()
    b1_d = nc.dram_tensor("b1", [E, F], FP32, kind="ExternalInput").ap()
    w2_d = nc.dram_tensor("W2f8", [E, F, D], F8E4, kind="ExternalInput").ap()
    b2_d = nc.dram_tensor("b2", [E, D], FP32, kind="ExternalInput").ap()
    sel_d = nc.dram_tensor("sel8", [E, E * 128], FP32,
                           kind="ExternalInput").ap()
    aux1_d = nc.dram_tensor("aux_ones", [128, 128], FP32,
                            kind="ExternalInput").ap()
    auxb_d = nc.dram_tensor("aux_ones_bf", [128, 128], BF16,
                            kind="ExternalInput").ap()
    aux64_d = nc.dram_tensor("aux_ones64", [65, 128], FP32,
                             kind="ExternalInput").ap()
    auxe_d = nc.dram_tensor("aux_eps", [1, 1], FP32,
                            kind="ExternalInput").ap()
    out_d = nc.dram_tensor("out", [T, D], FP32, kind="ExternalOutput").ap()

    with tile.TileContext(nc) as tc, ExitStack() as top:
        const = top.enter_context(tc.tile_pool(name="const", bufs=1))
        vecs = top.enter_context(tc.tile_pool(name="vecs", bufs=1))
        rows = top.enter_context(tc.tile_pool(name="rows", bufs=2))
        psA = top.enter_context(tc.tile_pool(name="psA", bufs=4, space="PSUM"))
        psB = top.enter_context(tc.tile_pool(name="psB", bufs=4, space="PSUM"))
        tmp = top.enter_context(tc.tile_pool(name="tmp", bufs=2))
        persist = top.enter_context(tc.tile_pool(name="persist", bufs=1))

        def pa(p, f):
            return psA.tile([p, f], FP32, tag="a", name="pa")

        def pb(p, f):
            return psB.tile([p, f], FP32, tag="b", name="pb")

        # ---------------- constants ----------------
        ident = const.tile([128, 128], FP32, tag="ident")
        make_identity(nc, ident)
        ones_col_r = const.tile([128, 1], FP32R, tag="ones_col_r")
        nc.gpsimd.dma_start(out=ones_col_r[:], in_=aux1_d[:, 0:1])
        ones_row_r = const.tile([1, 128], FP32R, tag="ones_row_r")
        nc.gpsimd.dma_start(out=ones_row_r[:], in_=aux1_d[0:1, :])
        ones_row8_r = const.tile([1, 8], FP32R, tag="ones_row8_r")
        nc.gpsimd.dma_start(out=ones_row8_r[:], in_=aux1_d[0:1, 0:8])
        ones8_col = const.tile([8, 1], FP32, tag="ones8_col")
        nc.gpsimd.dma_start(out=ones8_col[:], in_=aux1_d[0:8, 0:1])
        # row 64 all-ones: lhsT for the 1/s broadcast (base matches pctx[64])
        ones64r = const.tile([65, 128], FP32R, tag="ones64r")
        nc.gpsimd.dma_start(out=ones64r[:], in_=aux64_d[:, :])
        eps_t = const.tile([1, 1], FP32, tag="eps")
        nc.gpsimd.dma_start(out=eps_t[:], in_=auxe_d[:, :])
        # per-expert selector: sel8[i, e*128+p] = (i==e)/256 (fp8 descale)
        sel8 = const.tile([8, E * 128], FP32R, tag="sel8")
        nc.gpsimd.dma_start(out=sel8[:], in_=sel_d[:, :])

        def load_col(dvec, nb, dtype=FP32, tag=None):
            # [nb*128] DRAM vector -> [128, nb] feature-major column tile
            # (SWDGE ring: keeps the HWDGE rings clear for the big loads)
            raw = rows.tile([nb, 128], FP32, tag="rawvec")
            nc.scalar.dma_start(out=raw[:],
                                in_=dvec.rearrange("(a b) -> a b", b=128))
            ps = pb(128, nb)
            nc.tensor.transpose(ps[:], raw[:], ident[:nb, :nb])
            col = vecs.tile([128, nb], dtype, tag=tag)
            nc.vector.tensor_copy(col[:], ps[:])
            return col

        wgs = vecs.tile([128, DK, E], FP32R, tag="wg")
        nc.gpsimd.dma_start(
            out=wgs[:], in_=wg_d.rearrange("(kb p) e -> p kb e", p=128))
        b2s = vecs.tile([E, D], FP32R, tag="b2")
        nc.gpsimd.dma_start(out=b2s[:], in_=b2_d[:, :])

        # persistent activations (full T)
        x_t = [persist.tile([128, T], FP32R, tag=f"xt{k}", name=f"xt{k}")
               for k in range(DK)]
        x8 = [persist.tile([128, 2, T], F8E4, tag=f"x8_{dp}", name=f"x8_{dp}")
              for dp in range(3)]
        moe = [persist.tile([128, T], FP32, tag=f"moe{k}", name=f"moe{k}")
               for k in range(DK)]
        bias_total = vecs.tile([128, DK], FP32, tag="btot")

        def layer_norm(jobs):
            # feature-major LN over D=768 partitions (6 tiles), fp32r input.
            # jobs: list of (r_tiles, g_col, b_col, out_tiles, in_off,
            # out_off, nl) — stage-major across jobs so one job's serial
            # stats chain overlaps the other's matmuls/evictions.
            st = []
            for (r_tiles, g_col, b_col, out_tiles, n0, o0, nl) in jobs:
                ps_s = pa(1, TH)
                ps_s2 = pa(1, TH)
                for k in range(DK):
                    sq = tmp.tile([128, TH], FP32R, tag="ln_sq", bufs=3)
                    nc.scalar.activation(sq[:], r_tiles[k][:, n0:n0 + nl],
                                         AF.Square)
                    nc.tensor.matmul(ps_s[:], ones_col_r[:],
                                     r_tiles[k][:, n0:n0 + nl],
                                     start=(k == 0), stop=(k == DK - 1))
                    nc.tensor.matmul(ps_s2[:], ones_col_r[:], sq[:],
                                     start=(k == 0), stop=(k == DK - 1))
                st.append((ps_s, ps_s2))
            br = []
            for ji, (r_tiles, g_col, b_col, out_tiles, n0, o0, nl) in \
                    enumerate(jobs):
                ps_s, ps_s2 = st[ji]
                m = rows.tile([1, TH], FP32, tag="ln_m", bufs=2)
                m2 = rows.tile([1, TH], FP32, tag="ln_m2", bufs=2)
                nc.vector.tensor_scalar_mul(m[:], ps_s[:], 1.0 / D)
                nc.vector.tensor_scalar_mul(m2[:], ps_s2[:], 1.0 / D)
                mm_ = rows.tile([1, TH], FP32, tag="ln_mm", bufs=2)
                nc.vector.tensor_mul(mm_[:], m[:], m[:])
                var = rows.tile([1, TH], FP32, tag="ln_var", bufs=2)
                nc.vector.tensor_sub(var[:], m2[:], mm_[:])
                sd = rows.tile([1, TH], FP32, tag="ln_sd", bufs=2)
                nc.scalar.activation(sd[:], var[:], AF.Sqrt, bias=eps_t[:])
                rstd = rows.tile([1, TH], FP32R, tag="ln_rstd", bufs=2)
                with nc.allow_low_precision(reason="fp32r matmul operand"):
                    nc.vector.reciprocal(rstd[:], sd[:])
                mr = rows.tile([1, TH], FP32R, tag="ln_mr", bufs=2)
                nc.vector.tensor_mul(mr[:], m[:], rstd[:])
                pR = pb(128, TH)
                nc.tensor.matmul(pR[:], ones_row_r[:], rstd[:],
                                 start=True, stop=True)
                pM = pb(128, TH)
                nc.tensor.matmul(pM[:], ones_row_r[:], mr[:],
                                 start=True, stop=True)
                br.append((pR, pM))
            for ji, (r_tiles, g_col, b_col, out_tiles, n0, o0, nl) in \
                    enumerate(jobs):
                pR, pM = br[ji]
                for k in range(DK):
                    t1 = tmp.tile([128, TH], FP32, tag="ln_t1")
                    nc.vector.tensor_mul(t1[:], r_tiles[k][:, n0:n0 + nl],
                                         pR[:])
                    t2 = tmp.tile([128, TH], FP32, tag="ln_t2")
                    nc.vector.tensor_sub(t2[:], t1[:], pM[:])
                    nc.scalar.activation(out_tiles[k][:, o0:o0 + nl],
                                         t2[:], AF.Identity,
                                         bias=b_col[:, k:k + 1],
                                         scale=g_col[:, k:k + 1])

        # ================= attention =================
        with ExitStack() as hs:
            paw = hs.enter_context(tc.tile_pool(name="paw", bufs=3))
            pq = hs.enter_context(tc.tile_pool(name="pq", bufs=1))
            pproj = hs.enter_context(tc.tile_pool(name="pproj", bufs=1))
            phv = hs.enter_context(tc.tile_pool(name="phv", bufs=8))
            kvs = ExitStack()
            pkv = kvs.enter_context(tc.tile_pool(name="pkv", bufs=6))

            def load_w(dram):
                # 3 slots: wq/wk/wv stream back-to-back; wo reuses wq's
                wt = paw.tile([128, DK, D], BF16, tag="w", name="w")
                nc.scalar.dma_start(
                    out=wt[:], in_=dram.rearrange("(kb p) d -> p kb d", p=128))
                return wt

            # ---- q feature-major via xbar transpose-DMA, then qh ----
            wq = load_w(wq_d)
            q_t = [pq.tile([128, TP], BF16, tag=f"qt{k}", name=f"qt{k}")
                   for k in range(DK)]
            for k in range(DK):
                nc.sync.dma_start(out=q_t[k][:], in_=q_d[k],
                                  transpose=True)
            # column-vector loads ride the scalar ring behind wq, ready
            # before the first eviction needs them
            bq_col = load_col(bq_d, DK, tag="bq")
            bk_col = load_col(bk_d, DK, tag="bk")
            bo_col = load_col(bo_d, DK, tag="bo")
            bv_col = load_col(bv_d, DK, BF16, tag="bv")
            l1g_col = load_col(l1g_d, DK, tag="l1g")
            l1b_col = load_col(l1b_d, DK, tag="l1b")
            l2g_col = load_col(l2g_d, DK, tag="l2g")
            l2b_col = load_col(l2b_d, DK, tag="l2b")
            bg_col = vecs.tile([8, 1], FP32, tag="bg")
            nc.scalar.dma_start(out=bg_col[:],
                                in_=bg_d.rearrange("(a b) -> a b", b=1))

            qh_t = [pproj.tile([128, T], BF16, tag=f"qh{k}", name=f"qh{k}")
                    for k in range(DK)]
            kh_t = [pproj.tile([128, T], BF16, tag=f"kh{k}", name=f"kh{k}")
                    for k in range(DK)]

            def project(w, src, dst, bcol, scope, mi_list=None):
                with nc.named_scope(scope):
                    for (n0, nl) in NCH:
                        for mi in (mi_list or range(DK)):
                            ps = pa(128, TH)
                            for k in range(DK):
                                nc.tensor.matmul(
                                    ps[:], w[:, k, mi * 128:(mi + 1) * 128],
                                    src[k][:, n0:n0 + nl],
                                    start=(k == 0), stop=(k == DK - 1))
                            nc.scalar.activation(dst[mi][:, n0:n0 + nl],
                                                 ps[:], AF.Identity,
                                                 bias=bcol[:, mi:mi + 1])

            project(wq, q_t, qh_t, bq_col, "proj_q", mi_list=[0, 1, 2])

            # ---- k, kh ----
            wk = load_w(wk_d)
            k_t = [pkv.tile([128, TP], BF16, tag="kv", name=f"kt{k}")
                   for k in range(DK)]
            for k in range(DK):
                nc.sync.dma_start(out=k_t[k][:], in_=k_d[k],
                                  transpose=True)
            project(wk, k_t, kh_t, bk_col, "proj_k")

            # ---- v, vh (all 4 batches), token-major, ones col ----
            wv = load_w(wv_d)
            v_t = [pkv.tile([128, TP], BF16, tag="kv", name=f"vt{k}")
                   for k in range(DK)]
            for k in range(DK):
                nc.sync.dma_start(out=v_t[k][:], in_=v_d[k],
                                  transpose=True)
            vh = {}
            with nc.named_scope("vh"):
                for bt in range(4):
                    for ci, (c0, cl) in enumerate(BK):
                        vt_ = phv.tile([128, H, DH + 1], BF16,
                                       tag="vh", name=f"vh{bt}{ci}")
                        nc.scalar.dma_start(out=vt_[:cl, :, DH:DH + 1],
                                            in_=auxb_d[:cl, 0:H])
                        tc0 = bt * S + c0
                        for ni in range(2):
                            ps = pa(128, 384)
                            for k in range(DK):
                                nc.tensor.matmul(
                                    ps[:cl, :], v_t[k][:, tc0:tc0 + cl],
                                    wv[:, k, ni * 384:(ni + 1) * 384],
                                    start=(k == 0), stop=(k == DK - 1))
                            nc.vector.tensor_copy(
                                vt_[:cl, ni * 6:(ni + 1) * 6, 0:DH],
                                ps[:cl, :].rearrange("p (h d) -> p h d",
                                                     d=DH))
                        vh[(bt, ci)] = vt_

            # k_t/v_t dead past here — release their SBUF before the
            # head-loop pools open
            kvs.close()

            with ExitStack() as ph_:
                phe = ph_.enter_context(tc.tile_pool(name="phe", bufs=4))
                pho = ph_.enter_context(tc.tile_pool(name="pho", bufs=2))
                pcx = ph_.enter_context(tc.tile_pool(name="pcx", bufs=1))

                # ---- attention, batch-pair packed (N=392, bf16) ----
                # Both halves' head chains interleave: while one half's
                # softmax tail drains, the other half's scores/ctx matmuls
                # keep the PE fed.
                cxp2 = [[pcx.tile([128, TH], BF16, tag=f"cx{half}{mi}",
                                  name=f"cx{half}{mi}") for mi in range(DK)]
                        for half in range(2)]
                with nc.named_scope("heads"):
                    for hh in range(H):
                        if hh in (1, 2, 3):
                            project(wq, q_t, qh_t, bq_col, "proj_fill",
                                    mi_list=[hh + 2])
                        dm, ro = divmod(hh * DH, 128)
                        for half in range(2):
                            h0tok = half * TH
                            cxp = cxp2[half]
                            pctxs = []
                            for bl in range(2):
                                bq0 = h0tok + bl * S
                                # both ki-chunks' scores^T share one psum
                                # bank (196-col slices at 256 offsets) so
                                # ONE exp call covers them
                                ps = psA.tile([128, 2, 256], FP32, tag="a",
                                              name="psc")
                                for ci, (c0, cl) in enumerate(BK):
                                    nc.tensor.matmul(
                                        ps[:cl, ci, 0:S],
                                        kh_t[dm][ro:ro + DH,
                                                 bq0 + c0:bq0 + c0 + cl],
                                        qh_t[dm][ro:ro + DH, bq0:bq0 + S],
                                        start=True, stop=True)
                                ex = phe.tile([128, 2, S], BF16, tag="exp",
                                              bufs=4)
                                nc.scalar.activation(ex[:], ps[:, :, 0:S],
                                                     AF.Exp, scale=0.125)
                                pctx = pb(DH + 1, S)
                                for ci, (c0, cl) in enumerate(BK):
                                    nc.tensor.matmul(
                                        pctx[:],
                                        vh[(half * 2 + bl, ci)][:cl, hh, :],
                                        ex[:cl, ci, :],
                                        start=(ci == 0), stop=(ci == 1))
                                pctxs.append(pctx)
                            srec = rows.tile([65, TH], FP32R, tag="srec",
                                             bufs=3)
                            with nc.allow_low_precision(reason="fp32r rep"):
                                nc.vector.reciprocal(srec[64:65, 0:S],
                                                     pctxs[0][64:65, 0:S])
                                nc.vector.reciprocal(srec[64:65, S:TH],
                                                     pctxs[1][64:65, 0:S])
                            prep = pb(DH, TH)
                            nc.tensor.matmul(prep[:], ones64r[64:65, 0:DH],
                                             srec[64:65, :],
                                             start=True, stop=True)
                            prs = phe.tile([64, TH], BF16, tag="prs", bufs=2)
                            nc.vector.tensor_copy(prs[:], prep[:])
                            for bl in range(2):
                                bc = bl * S
                                if ro == 0:
                                    nc.vector.tensor_mul(
                                        cxp[dm][0:DH, bc:bc + S],
                                        pctxs[bl][0:DH, 0:S],
                                        prs[:, bc:bc + S])
                                else:
                                    co = pho.tile([64, S], BF16, tag="cxodd")
                                    nc.vector.tensor_mul(
                                        co[:], pctxs[bl][0:DH, 0:S],
                                        prs[:, bc:bc + S])
                                    nc.scalar.dma_start(
                                        out=cxp[dm][64:128, bc:bc + S],
                                        in_=co[:])

                # ---- Wo + bias_total = Wo^T bv + bo ----
                wo = load_w(wo_d)
                for mi in range(DK):
                    pbs = pb(128, 1)
                    for k in range(DK):
                        nc.tensor.matmul(pbs[:],
                                         wo[:, k, mi * 128:(mi + 1) * 128],
                                         bv_col[:, k:k + 1],
                                         start=(k == 0), stop=(k == DK - 1))
                    nc.vector.tensor_add(bias_total[:, mi:mi + 1], pbs[:],
                                         bo_col[:, mi:mi + 1])

                # all experts' b1, feature-major [128, E, FK] — PE work here
                # fills the LN1 stats-chain bubbles
                b1c = vecs.tile([128, E, FK], FP32, tag="b1c")
                for e in range(E):
                    braw = rows.tile([FK, 128], FP32, tag="rawb1")
                    nc.sync.dma_start(
                        out=braw[:],
                        in_=b1_d[e].rearrange("(a b) -> a b", b=128))
                    pbv = pb(128, FK)
                    nc.tensor.transpose(pbv[:], braw[:], ident[:FK, :FK])
                    nc.vector.tensor_copy(b1c[:, e, :], pbv[:])

                # ---- Wo projection + residual -> r1, LN1 -> x, x8 ----
                with nc.named_scope("wo_ln"):
                    r1h = [[pcx.tile([128, TH], FP32R, tag=f"r1{half}{mi}",
                                     name=f"r1{half}{mi}")
                            for mi in range(DK)] for half in range(2)]
                    for mi in range(DK):
                        for half in range(2):
                            h0tok = half * TH
                            ps = pa(128, TH)
                            for k in range(DK):
                                nc.tensor.matmul(
                                    ps[:], wo[:, k, mi * 128:(mi + 1) * 128],
                                    cxp2[half][k][:],
                                    start=(k == 0), stop=(k == DK - 1))
                            nc.vector.scalar_tensor_tensor(
                                out=r1h[half][mi][:], in0=ps[:],
                                scalar=bias_total[:, mi:mi + 1],
                                in1=q_t[mi][:, h0tok:h0tok + TH],
                                op0=OP.add, op1=OP.add)

                    layer_norm([(r1h[0], l1g_col, l1b_col, x_t, 0, 0, TH),
                                (r1h[1], l1g_col, l1b_col, x_t, 0, TH, TH)])
                    for half in range(2):
                        h0tok = half * TH
                        for dp in range(3):
                            for i in range(2):
                                nc.vector.tensor_copy(
                                    x8[dp][:, i, h0tok:h0tok + TH],
                                    x_t[2 * dp + i][:, h0tok:h0tok + TH])

        # ================= gates =================
        gexp = persist.tile([8, T], FP32, tag="gexp")
        gate = persist.tile([8, T], FP32R, tag="gate")
        with nc.named_scope("gates"):
            # stage-major over the two token halves: one half's serial
            # softmax chain overlaps the other's matmuls
            pgl = []
            for (n0, nl) in NCH:
                pg = pb(8, TH)
                for k in range(DK):
                    nc.tensor.matmul(pg[:], wgs[:, k, :],
                                     x_t[k][:, n0:n0 + nl],
                                     start=(k == 0), stop=(k == DK - 1))
                nc.scalar.activation(gexp[:, n0:n0 + nl], pg[:], AF.Exp,
                                     bias=bg_col[:])
                pgl.append(pg)
            for ci, (n0, nl) in enumerate(NCH):
                pgs = pa(1, TH)
                nc.tensor.matmul(pgs[:], ones8_col[:], gexp[:, n0:n0 + nl],
                                 start=True, stop=True)
                grec = rows.tile([1, TH], FP32R, tag="grec", bufs=2)
                with nc.allow_low_precision(reason="fp32r matmul operand"):
                    nc.vector.reciprocal(grec[:], pgs[:])
                pgr = pb(8, TH)
                nc.tensor.matmul(pgr[:], ones_row8_r[:], grec[:],
                                 start=True, stop=True)
                nc.vector.tensor_mul(gate[:, n0:n0 + nl],
                                     gexp[:, n0:n0 + nl], pgr[:])

            # moe_acc init = gates^T @ b2   (lhsT = b2 chunks [8, 128])
            for mi in range(DK):
                for (n0, nl) in NCH:
                    pbi = pa(128, TH)
                    nc.tensor.matmul(pbi[:], b2s[:, mi * 128:(mi + 1) * 128],
                                     gate[:, n0:n0 + nl],
                                     start=True, stop=True)
                    nc.scalar.copy(moe[mi][:, n0:n0 + nl], pbi[:])

        # ================= MoE experts (fp8 DoubleRow) =================
        with ExitStack() as ms:
            pmh = ms.enter_context(tc.tile_pool(name="pmh", bufs=26))
            pmw1 = ms.enter_context(tc.tile_pool(name="pmw1", bufs=4))
            pmw2 = ms.enter_context(tc.tile_pool(name="pmw2", bufs=16))
            for e in range(E):
              with nc.named_scope(f"moe{e}"):
                # gate row broadcast to 128 partitions (carries 1/256 descale)
                grep = tmp.tile([128, T], BF16, tag="gerep")
                for (n0, nl) in NCH:
                    pge = pb(128, TH)
                    nc.tensor.matmul(pge[:],
                                     sel8[:, e * 128:(e + 1) * 128],
                                     gate[:, n0:n0 + nl],
                                     start=True, stop=True)
                    nc.vector.tensor_copy(grep[:, n0:n0 + nl], pge[:])

                # expert weights, fp8, DoubleRow pair layout
                w1t = []
                for dp in range(3):
                    wt = pmw1.tile([128, 2, F], F8E4, tag="w1", name="w1t")
                    nc.sync.dma_start(
                        out=wt[:],
                        in_=w1_d[e, dp * 256:(dp + 1) * 256, :].rearrange(
                            "(i p) f -> p i f", p=128))
                    w1t.append(wt)
                w2t = []
                for fbp in range(FK // 2):
                    wt = pmw2.tile([128, 2, D], F8E4, tag="w2", name="w2t")
                    nc.sync.dma_start(
                        out=wt[:],
                        in_=w2_d[e, fbp * 256:(fbp + 1) * 256, :].rearrange(
                            "(i p) d -> p i d", p=128))
                    w2t.append(wt)

                # ---- h = gelu((W1*256)^T x / 256 + b1) -> fp8 [F, T] ----
                hts = []
                for fm in range(FK):
                    fbp, ih = divmod(fm, 2)
                    if ih == 0:
                        hp = pmh.tile([128, 2, T], F8E4, tag="h", name="hp")
                        hts.append(hp)
                    ph0 = pa(128, TH)
                    ph1 = pb(128, TH)
                    for dp in range(3):
                        nc.tensor.matmul(
                            ph0[:], w1t[dp][:, :, fm * 128:(fm + 1) * 128],
                            x8[dp][:, :, 0:TH],
                            start=(dp == 0), stop=(dp == 2), perf_mode=DR)
                        nc.tensor.matmul(
                            ph1[:], w1t[dp][:, :, fm * 128:(fm + 1) * 128],
                            x8[dp][:, :, TH:T],
                            start=(dp == 0), stop=(dp == 2), perf_mode=DR)
                    nc.scalar.activation(hts[fbp][:, ih, 0:TH], ph0[:],
                                         AF.Gelu, bias=b1c[:, e, fm:fm + 1],
                                         scale=1.0 / W8SCALE)
                    nc.scalar.activation(hts[fbp][:, ih, TH:T], ph1[:],
                                         AF.Gelu, bias=b1c[:, e, fm:fm + 1],
                                         scale=1.0 / W8SCALE)

                # ---- y = (W2*256)^T h (K-accum in PSUM), combine ----
                for dg in range(3):
                    pys = [pa(128, TH) for _ in range(2)] + \
                          [pb(128, TH) for _ in range(2)]
                    for fbp in range(FK // 2):
                        for j in range(2):
                            m0 = dg * 256 + j * 128
                            for ni, (n0, nl) in enumerate(NCH):
                                nc.tensor.matmul(
                                    pys[j * 2 + ni][:],
                                    w2t[fbp][:, :, m0:m0 + 128],
                                    hts[fbp][:, :, n0:n0 + nl],
                                    start=(fbp == 0), stop=(fbp == 11),
                                    perf_mode=DR)
                    for j in range(2):
                        mi = dg * 2 + j
                        for ni, (n0, nl) in enumerate(NCH):
                            ty = tmp.tile([128, TH], FP32, tag="ty")
                            nc.vector.tensor_mul(ty[:], pys[j * 2 + ni][:],
                                                 grep[:, n0:n0 + nl])
                            nc.vector.tensor_add(moe[mi][:, n0:n0 + nl],
                                                 moe[mi][:, n0:n0 + nl],
                                                 ty[:])

        # ================= LN2 + output =================
        with ExitStack() as fs:
            pfo = fs.enter_context(tc.tile_pool(name="pfo", bufs=3))
            with nc.named_scope("tail"):
                for (n0, nl) in NCH:
                    for mi in range(DK):
                        nc.vector.tensor_add(x_t[mi][:, n0:n0 + nl],
                                             x_t[mi][:, n0:n0 + nl],
                                             moe[mi][:, n0:n0 + nl])
                layer_norm(
                    [(x_t, l2g_col, l2b_col, moe, n0, n0, nl)
                     for (n0, nl) in NCH])
                for ci in range(2):
                    for (t0, tl) in TTH[ci]:
                        ot = pfo.tile([128, D], FP32, tag="otok")
                        for k in range(DK):
                            ps = pa(128, 128)
                            nc.tensor.transpose(ps[:tl, :],
                                                moe[k][:, t0:t0 + tl],
                                                ident[:, :])
                            nc.vector.tensor_copy(
                                ot[:tl, k * 128:(k + 1) * 128], ps[:tl, :])
                        oeng = nc.sync if (t0 // 128) % 2 == 0 else nc.scalar
                        oeng.dma_start(out=out_d[t0:t0 + tl, :],
                                       in_=ot[:tl, :])

    nc.compile()
    return nc


def _get_nc():
    if "nc" not in _CACHE:
        _CACHE["nc"] = _build()
    return _CACHE["nc"]


def run(inputs, **spmd_kwargs):
    nc = _get_nc()
    f32 = np.float32
    bf16 = ml_dtypes.bfloat16
    f8 = ml_dtypes.float8_e4m3
    inp = {k: np.asarray(v) for k, v in inputs.items()}
    shared = {}
    for name in ("bq", "bk", "bv", "bo", "ln1_g", "ln1_b", "ln2_g", "ln2_b",
                 "Wg", "bg", "b1", "b2"):
        shared[name] = np.ascontiguousarray(inp[name].astype(f32))
    for name in ("Wq", "Wk", "Wv", "Wo"):
        shared[name] = np.ascontiguousarray(inp[name].astype(f32).astype(bf16))
    shared["W1f8"] = np.ascontiguousarray(
        (inp["W1"].astype(f32) * W8SCALE).astype(f8))
    shared["W2f8"] = np.ascontiguousarray(
        (inp["W2"].astype(f32) * W8SCALE).astype(f8))
    sel = np.zeros((E, E * 128), dtype=f32)
    for e in range(E):
        sel[e, e * 128:(e + 1) * 128] = 1.0 / W8SCALE
    shared["sel8"] = sel
    shared["aux_ones"] = np.ones((128, 128), dtype=f32)
    shared["aux_ones_bf"] = np.ones((128, 128), dtype=bf16)
    a64 = np.zeros((65, 128), dtype=f32)
    a64[64, :] = 1.0
    shared["aux_ones64"] = a64
    shared["aux_eps"] = np.full((1, 1), EPS, dtype=f32)
    in_maps = []
    for c in range(NCORES):
        m = dict(shared)
        for name in ("q", "k", "v"):
            pad = np.zeros((TP, D), dtype=bf16)
            pad[:T] = inp[name][c * BPC:(c + 1) * BPC].reshape(
                T, D).astype(f32).astype(bf16)
            # block-major [DK, TP, 128]: each xbar transpose-DMA reads one
            # fully contiguous region
            m[name] = np.ascontiguousarray(
                pad.reshape(TP, DK, 128).transpose(1, 0, 2))
        in_maps.append(m)
    res = run_bass_kernel_spmd(nc, in_maps, core_ids=list(range(NCORES)),
                               **spmd_kwargs)
    out = np.stack([r["out"] for r in res.results])  # [8, T, D]
    return out.reshape(B, S, D), res


def kernel(**inputs):
    out, _ = run(inputs)
    return out


# revision 39
# speedup vs baseline: 1.0701x; 1.0169x over previous
"""MoE transformer block on 8 TRN2 NeuronCores.

Data-parallel over batch (4 batches = 784 tokens per core), no
collectives.  ~738 us HW exec (from a 1255 us fp32r baseline), max rel
err ~9e-3 vs the fp32 reference.

- Attention in bf16: Wq/Wk/Wv/Wo host-cast to bf16 (ACT HWDGE ring);
  q/k/v host-padded to 896 rows, cast bf16, laid out block-major
  [DK, 896, 128] so each xbar transpose-DMA (SP ring) reads one
  contiguous region; no PE transposes on the load path.
- Scores/softmax/ctx per (head, batch) at N=196 — bf16 matmuls have no
  min-free-dim constraint, so no batch-pair packing of the query dim.
  Both token halves' head chains interleave inside one loop so their
  serial softmax tails overlap.
- MoE FFNs in fp8e4m3 with DoubleRow matmuls (256-row contraction per
  pass, ~2x PE throughput): W1/W2 host-scaled by 256 and cast; x
  re-quantized to fp8 after LN1; h = gelu(psum/256 + b1) evicts
  straight to fp8; the 1/256 descale of the W2 product rides in the
  sel8 gate-broadcast selector.
- LayerNorm (stage-major across both token halves), softmax
  normalization, and gating stay in fp32/fp32r.

PSUM discipline: two pools, one unified tag each (every psum tile <= 1
bank, 4 bufs per pool -> exactly 8 banks).  The MoE y-phase holds 2+2
accumulators across the K(=F/2) loop while the next expert's h-phase
double-buffers 1+1.
"""
import sys

sys.path.insert(0, "/opt/trn_rl_repo")

from contextlib import ExitStack

import ml_dtypes
import numpy as np

import concourse.bass as bass
import concourse.tile as tile
from concourse import bacc, mybir
from concourse.bass_utils import run_bass_kernel_spmd
from concourse.masks import make_identity

FP32 = mybir.dt.float32
FP32R = mybir.dt.float32r
BF16 = mybir.dt.bfloat16
F8E4 = mybir.dt.float8e4
AF = mybir.ActivationFunctionType
OP = mybir.AluOpType
DR = mybir.MatmulPerfMode.DoubleRow

B, S, D, H, E, F = 32, 196, 768, 12, 8, 3072
DH = D // H                 # 64
NCORES = 8
BPC = B // NCORES           # 4 batches per core
T = BPC * S                 # 784 tokens per core
TP = 896                    # padded token count (multiple of 128) for xbar
TH = T // 2                 # 392 tokens per half (2 batches)
DK = D // 128               # 6
FK = F // 128               # 24
EPS = 1e-5
W8SCALE = 256.0             # host-side fp8 weight scale
BK = [(0, 128), (128, 68)]                              # ki chunks per batch
NCH = ((0, TH), (TH, TH))                               # token halves
# output token tiles, grouped by LN2 half
TTH = ([(0, 128), (128, 128), (256, 128), (384, 8)],
       [(392, 128), (520, 128), (648, 128), (776, 8)])

_CACHE = {}


def _build():
    nc = bacc.Bacc("TRN2", target_bir_lowering=False, debug=False,
                   num_devices=NCORES)

    q_d = nc.dram_tensor("q", [DK, TP, 128], BF16, kind="ExternalInput").ap()
    k_d = nc.dram_tensor("k", [DK, TP, 128], BF16, kind="ExternalInput").ap()
    v_d = nc.dram_tensor("v", [DK, TP, 128], BF16, kind="ExternalInput").ap()
    wq_d = nc.dram_tensor("Wq", [D, D], BF16, kind="ExternalInput").ap()
    wk_d = nc.dram_tensor("Wk", [D, D], BF16, kind="ExternalInput").ap()
    wv_d = nc.dram_tensor("Wv", [D, D], BF16, kind="ExternalInput").ap()
    wo_d = nc.dram_tensor("Wo", [D, D], BF16, kind="ExternalInput").ap()
    bq_d = nc.dram_tensor("bq", [D], FP32, kind="ExternalInput").ap()
    bk_d = nc.dram_tensor("bk", [D], FP32, kind="ExternalInput").ap()
    bv_d = nc.dram_tensor("bv", [D], FP32, kind="ExternalInput").ap()
    bo_d = nc.dram_tensor("bo", [D], FP32, kind="ExternalInput").ap()
    l1g_d = nc.dram_tensor("ln1_g", [D], FP32, kind="ExternalInput").ap()
    l1b_d = nc.dram_tensor("ln1_b", [D], FP32, kind="ExternalInput").ap()
    l2g_d = nc.dram_tensor("ln2_g", [D], FP32, kind="ExternalInput").ap()
    l2b_d = nc.dram_tensor("ln2_b", [D], FP32, kind="ExternalInput").ap()
    wg_d = nc.dram_tensor("Wg", [D, E], FP32, kind="ExternalInput").ap()
    bg_d = nc.dram_tensor("bg", [E], FP32, kind="ExternalInput").ap()
    w1_d = nc.dram_tensor("W1f8", [E, D, F], F8E4, kind="ExternalInput").ap()
    b1_d = nc.dram_tensor("b1", [E, F], FP32, kind="ExternalInput").ap()
    w2_d = nc.dram_tensor("W2f8", [E, F, D], F8E4, kind="ExternalInput").ap()
    b2_d = nc.dram_tensor("b2", [E, D], FP32, kind="ExternalInput").ap()
    sel_d = nc.dram_tensor("sel8", [E, E * 128], FP32,
                           kind="ExternalInput").ap()
    aux1_d = nc.dram_tensor("aux_ones", [128, 128], FP32,
                            kind="ExternalInput").ap()
    auxb_d = nc.dram_tensor("aux_ones_bf", [128, 128], BF16,
                            kind="ExternalInput").ap()
    aux64_d = nc.dram_tensor("aux_ones64", [65, 128], FP32,
                             kind="ExternalInput").ap()
    auxe_d = nc.dram_tensor("aux_eps", [1, 1], FP32,
                            kind="ExternalInput").ap()
    out_d = nc.dram_tensor("out", [T, D], FP32, kind="ExternalOutput").ap()

    with tile.TileContext(nc) as tc, ExitStack() as top:
        const = top.enter_context(tc.tile_pool(name="const", bufs=1))
        vecs = top.enter_context(tc.tile_pool(name="vecs", bufs=1))
        rows = top.enter_context(tc.tile_pool(name="rows", bufs=2))
        psA = top.enter_context(tc.tile_pool(name="psA", bufs=4, space="PSUM"))
        psB = top.enter_context(tc.tile_pool(name="psB", bufs=4, space="PSUM"))
        tmp = top.enter_context(tc.tile_pool(name="tmp", bufs=2))
        persist = top.enter_context(tc.tile_pool(name="persist", bufs=1))

        def pa(p, f):
            return psA.tile([p, f], FP32, tag="a", name="pa")

        def pb(p, f):
            return psB.tile([p, f], FP32, tag="b", name="pb")

        # ---------------- constants ----------------
        ident = const.tile([128, 128], FP32, tag="ident")
        make_identity(nc, ident)
        ones_col_r = const.tile([128, 1], FP32R, tag="ones_col_r")
        nc.gpsimd.dma_start(out=ones_col_r[:], in_=aux1_d[:, 0:1])
        ones_row_r = const.tile([1, 128], FP32R, tag="ones_row_r")
        nc.gpsimd.dma_start(out=ones_row_r[:], in_=aux1_d[0:1, :])
        ones_row8_r = const.tile([1, 8], FP32R, tag="ones_row8_r")
        nc.gpsimd.dma_start(out=ones_row8_r[:], in_=aux1_d[0:1, 0:8])
        ones8_col = const.tile([8, 1], FP32, tag="ones8_col")
        nc.gpsimd.dma_start(out=ones8_col[:], in_=aux1_d[0:8, 0:1])
        # row 64 all-ones: lhsT for the 1/s broadcast (base matches pctx[64])
        ones64r = const.tile([65, 128], FP32R, tag="ones64r")
        nc.gpsimd.dma_start(out=ones64r[:], in_=aux64_d[:, :])
        eps_t = const.tile([1, 1], FP32, tag="eps")
        nc.gpsimd.dma_start(out=eps_t[:], in_=auxe_d[:, :])
        # per-expert selector: sel8[i, e*128+p] = (i==e)/256 (fp8 descale)
        sel8 = const.tile([8, E * 128], FP32R, tag="sel8")
        nc.gpsimd.dma_start(out=sel8[:], in_=sel_d[:, :])

        def load_col(dvec, nb, dtype=FP32, tag=None):
            # [nb*128] DRAM vector -> [128, nb] feature-major column tile
            # (SWDGE ring: keeps the HWDGE rings clear for the big loads)
            raw = rows.tile([nb, 128], FP32, tag="rawvec")
            nc.scalar.dma_start(out=raw[:],
                                in_=dvec.rearrange("(a b) -> a b", b=128))
            ps = pb(128, nb)
            nc.tensor.transpose(ps[:], raw[:], ident[:nb, :nb])
            col = vecs.tile([128, nb], dtype, tag=tag)
            nc.vector.tensor_copy(col[:], ps[:])
            return col

        wgs = vecs.tile([128, DK, E], FP32R, tag="wg")
        nc.gpsimd.dma_start(
            out=wgs[:], in_=wg_d.rearrange("(kb p) e -> p kb e", p=128))
        b2s = vecs.tile([E, D], FP32R, tag="b2")
        nc.gpsimd.dma_start(out=b2s[:], in_=b2_d[:, :])

        # persistent activations (full T)
        x_t = [persist.tile([128, T], FP32R, tag=f"xt{k}", name=f"xt{k}")
               for k in range(DK)]
        x8 = [persist.tile([128, 2, T], F8E4, tag=f"x8_{dp}", name=f"x8_{dp}")
              for dp in range(3)]
        moe = [persist.tile([128, T], FP32, tag=f"moe{k}", name=f"moe{k}")
               for k in range(DK)]
        bias_total = vecs.tile([128, DK], FP32, tag="btot")

        def layer_norm(jobs):
            # feature-major LN over D=768 partitions (6 tiles), fp32r input.
            # jobs: list of (r_tiles, g_col, b_col, out_tiles, in_off,
            # out_off, nl) — stage-major across jobs so one job's serial
            # stats chain overlaps the other's matmuls/evictions.
            st = []
            for (r_tiles, g_col, b_col, out_tiles, n0, o0, nl) in jobs:
                ps_s = pa(1, TH)
                ps_s2 = pa(1, TH)
                for k in range(DK):
                    sq = tmp.tile([128, TH], FP32R, tag="ln_sq", bufs=3)
                    nc.scalar.activation(sq[:], r_tiles[k][:, n0:n0 + nl],
                                         AF.Square)
                    nc.tensor.matmul(ps_s[:], ones_col_r[:],
                                     r_tiles[k][:, n0:n0 + nl],
                                     start=(k == 0), stop=(k == DK - 1))
                    nc.tensor.matmul(ps_s2[:], ones_col_r[:], sq[:],
                                     start=(k == 0), stop=(k == DK - 1))
                st.append((ps_s, ps_s2))
            br = []
            for ji, (r_tiles, g_col, b_col, out_tiles, n0, o0, nl) in \
                    enumerate(jobs):
                ps_s, ps_s2 = st[ji]
                m = rows.tile([1, TH], FP32, tag="ln_m", bufs=2)
                m2 = rows.tile([1, TH], FP32, tag="ln_m2", bufs=2)
                nc.vector.tensor_scalar_mul(m[:], ps_s[:], 1.0 / D)
                nc.vector.tensor_scalar_mul(m2[:], ps_s2[:], 1.0 / D)
                mm_ = rows.tile([1, TH], FP32, tag="ln_mm", bufs=2)
                nc.vector.tensor_mul(mm_[:], m[:], m[:])
                var = rows.tile([1, TH], FP32, tag="ln_var", bufs=2)
                nc.vector.tensor_sub(var[:], m2[:], mm_[:])
                sd = rows.tile([1, TH], FP32, tag="ln_sd", bufs=2)
                nc.scalar.activation(sd[:], var[:], AF.Sqrt, bias=eps_t[:])
                rstd = rows.tile([1, TH], FP32R, tag="ln_rstd", bufs=2)
                with nc.allow_low_precision(reason="fp32r matmul operand"):
                    nc.vector.reciprocal(rstd[:], sd[:])
                mr = rows.tile([1, TH], FP32R, tag="ln_mr", bufs=2)
                nc.vector.tensor_mul(mr[:], m[:], rstd[:])
                pR = pb(128, TH)
                nc.tensor.matmul(pR[:], ones_row_r[:], rstd[:],
                                 start=True, stop=True)
                pM = pb(128, TH)
                nc.tensor.matmul(pM[:], ones_row_r[:], mr[:],
                                 start=True, stop=True)
                br.append((pR, pM))
            for ji, (r_tiles, g_col, b_col, out_tiles, n0, o0, nl) in \
                    enumerate(jobs):
                pR, pM = br[ji]
                for k in range(DK):
                    t1 = tmp.tile([128, TH], FP32, tag="ln_t1")
                    nc.vector.tensor_mul(t1[:], r_tiles[k][:, n0:n0 + nl],
                                         pR[:])
                    t2 = tmp.tile([128, TH], FP32, tag="ln_t2")
                    nc.vector.tensor_sub(t2[:], t1[:], pM[:])
                    nc.scalar.activation(out_tiles[k][:, o0:o0 + nl],
                                         t2[:], AF.Identity,
                                         bias=b_col[:, k:k + 1],
                                         scale=g_col[:, k:k + 1])

        # ================= attention =================
        with ExitStack() as hs:
            paw = hs.enter_context(tc.tile_pool(name="paw", bufs=3))
            pq = hs.enter_context(tc.tile_pool(name="pq", bufs=1))
            pproj = hs.enter_context(tc.tile_pool(name="pproj", bufs=1))
            phv = hs.enter_context(tc.tile_pool(name="phv", bufs=8))
            kvs = ExitStack()
            pkv = kvs.enter_context(tc.tile_pool(name="pkv", bufs=6))

            def load_w(dram):
                # 3 slots: wq/wk/wv stream back-to-back; wo reuses wq's
                wt = paw.tile([128, DK, D], BF16, tag="w", name="w")
                nc.scalar.dma_start(
                    out=wt[:], in_=dram.rearrange("(kb p) d -> p kb d", p=128))
                return wt

            # ---- q feature-major via xbar transpose-DMA, then qh ----
            wq = load_w(wq_d)
            q_t = [pq.tile([128, TP], BF16, tag=f"qt{k}", name=f"qt{k}")
                   for k in range(DK)]
            for k in range(DK):
                nc.sync.dma_start(out=q_t[k][:], in_=q_d[k],
                                  transpose=True)
            # column-vector loads ride the scalar ring behind wq, ready
            # before the first eviction needs them
            bq_col = load_col(bq_d, DK, tag="bq")
            bk_col = load_col(bk_d, DK, tag="bk")
            bo_col = load_col(bo_d, DK, tag="bo")
            bv_col = load_col(bv_d, DK, BF16, tag="bv")
            l1g_col = load_col(l1g_d, DK, tag="l1g")
            l1b_col = load_col(l1b_d, DK, tag="l1b")
            l2g_col = load_col(l2g_d, DK, tag="l2g")
            l2b_col = load_col(l2b_d, DK, tag="l2b")
            bg_col = vecs.tile([8, 1], FP32, tag="bg")
            nc.scalar.dma_start(out=bg_col[:],
                                in_=bg_d.rearrange("(a b) -> a b", b=1))

            qh_t = [pproj.tile([128, T], BF16, tag=f"qh{k}", name=f"qh{k}")
                    for k in range(DK)]
            kh_t = [pproj.tile([128, T], BF16, tag=f"kh{k}", name=f"kh{k}")
                    for k in range(DK)]

            def project(w, src, dst, bcol, scope):
                with nc.named_scope(scope):
                    for (n0, nl) in NCH:
                        for mi in range(DK):
                            ps = pa(128, TH)
                            for k in range(DK):
                                nc.tensor.matmul(
                                    ps[:], w[:, k, mi * 128:(mi + 1) * 128],
                                    src[k][:, n0:n0 + nl],
                                    start=(k == 0), stop=(k == DK - 1))
                            nc.scalar.activation(dst[mi][:, n0:n0 + nl],
                                                 ps[:], AF.Identity,
                                                 bias=bcol[:, mi:mi + 1])

            project(wq, q_t, qh_t, bq_col, "proj_q")

            # ---- k, kh ----
            wk = load_w(wk_d)
            k_t = [pkv.tile([128, TP], BF16, tag="kv", name=f"kt{k}")
                   for k in range(DK)]
            for k in range(DK):
                nc.sync.dma_start(out=k_t[k][:], in_=k_d[k],
                                  transpose=True)
            project(wk, k_t, kh_t, bk_col, "proj_k")

            # ---- v, vh (all 4 batches), token-major, ones col ----
            wv = load_w(wv_d)
            v_t = [pkv.tile([128, TP], BF16, tag="kv", name=f"vt{k}")
                   for k in range(DK)]
            for k in range(DK):
                nc.sync.dma_start(out=v_t[k][:], in_=v_d[k],
                                  transpose=True)
            vh = {}
            with nc.named_scope("vh"):
                for bt in range(4):
                    for ci, (c0, cl) in enumerate(BK):
                        vt_ = phv.tile([128, H, DH + 1], BF16,
                                       tag="vh", name=f"vh{bt}{ci}")
                        nc.scalar.dma_start(out=vt_[:cl, :, DH:DH + 1],
                                            in_=auxb_d[:cl, 0:H])
                        tc0 = bt * S + c0
                        for ni in range(2):
                            ps = pa(128, 384)
                            for k in range(DK):
                                nc.tensor.matmul(
                                    ps[:cl, :], v_t[k][:, tc0:tc0 + cl],
                                    wv[:, k, ni * 384:(ni + 1) * 384],
                                    start=(k == 0), stop=(k == DK - 1))
                            nc.vector.tensor_copy(
                                vt_[:cl, ni * 6:(ni + 1) * 6, 0:DH],
                                ps[:cl, :].rearrange("p (h d) -> p h d",
                                                     d=DH))
                        vh[(bt, ci)] = vt_

            # k_t/v_t dead past here — release their SBUF before the
            # head-loop pools open
            kvs.close()

            # ---- Wo + bias_total = Wo^T bv + bo ----
            wo = load_w(wo_d)
            for mi in range(DK):
                pbs = pb(128, 1)
                for k in range(DK):
                    nc.tensor.matmul(pbs[:],
                                     wo[:, k, mi * 128:(mi + 1) * 128],
                                     bv_col[:, k:k + 1],
                                     start=(k == 0), stop=(k == DK - 1))
                nc.vector.tensor_add(bias_total[:, mi:mi + 1], pbs[:],
                                     bo_col[:, mi:mi + 1])

            with ExitStack() as ph_:
                phe = ph_.enter_context(tc.tile_pool(name="phe", bufs=4))
                pho = ph_.enter_context(tc.tile_pool(name="pho", bufs=2))
                pcx = ph_.enter_context(tc.tile_pool(name="pcx", bufs=1))

                # ---- attention, batch-pair packed (N=392, bf16) ----
                # Both halves' head chains interleave: while one half's
                # softmax tail drains, the other half's scores/ctx matmuls
                # keep the PE fed.
                cxp2 = [[pcx.tile([128, TH], BF16, tag=f"cx{half}{mi}",
                                  name=f"cx{half}{mi}") for mi in range(DK)]
                        for half in range(2)]
                with nc.named_scope("heads"):
                    for hh in range(H):
                        dm, ro = divmod(hh * DH, 128)
                        for half in range(2):
                            h0tok = half * TH
                            cxp = cxp2[half]
                            pctxs = []
                            for bl in range(2):
                                bq0 = h0tok + bl * S
                                # both ki-chunks' scores^T share one psum
                                # bank (196-col slices at 256 offsets) so
                                # ONE exp call covers them
                                ps = psA.tile([128, 2, 256], FP32, tag="a",
                                              name="psc")
                                for ci, (c0, cl) in enumerate(BK):
                                    nc.tensor.matmul(
                                        ps[:cl, ci, 0:S],
                                        kh_t[dm][ro:ro + DH,
                                                 bq0 + c0:bq0 + c0 + cl],
                                        qh_t[dm][ro:ro + DH, bq0:bq0 + S],
                                        start=True, stop=True)
                                ex = phe.tile([128, 2, S], BF16, tag="exp",
                                              bufs=4)
                                nc.scalar.activation(ex[:], ps[:, :, 0:S],
                                                     AF.Exp, scale=0.125)
                                pctx = pb(DH + 1, S)
                                for ci, (c0, cl) in enumerate(BK):
                                    nc.tensor.matmul(
                                        pctx[:],
                                        vh[(half * 2 + bl, ci)][:cl, hh, :],
                                        ex[:cl, ci, :],
                                        start=(ci == 0), stop=(ci == 1))
                                pctxs.append(pctx)
                            srec = rows.tile([65, TH], FP32R, tag="srec",
                                             bufs=3)
                            with nc.allow_low_precision(reason="fp32r rep"):
                                nc.vector.reciprocal(srec[64:65, 0:S],
                                                     pctxs[0][64:65, 0:S])
                                nc.vector.reciprocal(srec[64:65, S:TH],
                                                     pctxs[1][64:65, 0:S])
                            prep = pb(DH, TH)
                            nc.tensor.matmul(prep[:], ones64r[64:65, 0:DH],
                                             srec[64:65, :],
                                             start=True, stop=True)
                            prs = phe.tile([64, TH], BF16, tag="prs", bufs=2)
                            nc.vector.tensor_copy(prs[:], prep[:])
                            for bl in range(2):
                                bc = bl * S
                                if ro == 0:
                                    nc.vector.tensor_mul(
                                        cxp[dm][0:DH, bc:bc + S],
                                        pctxs[bl][0:DH, 0:S],
                                        prs[:, bc:bc + S])
                                else:
                                    co = pho.tile([64, S], BF16, tag="cxodd")
                                    nc.vector.tensor_mul(
                                        co[:], pctxs[bl][0:DH, 0:S],
                                        prs[:, bc:bc + S])
                                    nc.scalar.dma_start(
                                        out=cxp[dm][64:128, bc:bc + S],
                                        in_=co[:])

                # all experts' b1, feature-major [128, E, FK] — PE work here
                # fills the LN1 stats-chain bubbles
                b1c = vecs.tile([128, E, FK], FP32, tag="b1c")
                for e in range(E):
                    braw = rows.tile([FK, 128], FP32, tag="rawb1")
                    nc.sync.dma_start(
                        out=braw[:],
                        in_=b1_d[e].rearrange("(a b) -> a b", b=128))
                    pbv = pb(128, FK)
                    nc.tensor.transpose(pbv[:], braw[:], ident[:FK, :FK])
                    nc.vector.tensor_copy(b1c[:, e, :], pbv[:])

                # ---- Wo projection + residual -> r1, LN1 -> x, x8 ----
                with nc.named_scope("wo_ln"):
                    r1h = [[pcx.tile([128, TH], FP32R, tag=f"r1{half}{mi}",
                                     name=f"r1{half}{mi}")
                            for mi in range(DK)] for half in range(2)]
                    for mi in range(DK):
                        for half in range(2):
                            h0tok = half * TH
                            ps = pa(128, TH)
                            for k in range(DK):
                                nc.tensor.matmul(
                                    ps[:], wo[:, k, mi * 128:(mi + 1) * 128],
                                    cxp2[half][k][:],
                                    start=(k == 0), stop=(k == DK - 1))
                            nc.vector.scalar_tensor_tensor(
                                out=r1h[half][mi][:], in0=ps[:],
                                scalar=bias_total[:, mi:mi + 1],
                                in1=q_t[mi][:, h0tok:h0tok + TH],
                                op0=OP.add, op1=OP.add)

                    layer_norm([(r1h[0], l1g_col, l1b_col, x_t, 0, 0, TH),
                                (r1h[1], l1g_col, l1b_col, x_t, 0, TH, TH)])
                    for half in range(2):
                        h0tok = half * TH
                        for dp in range(3):
                            for i in range(2):
                                nc.vector.tensor_copy(
                                    x8[dp][:, i, h0tok:h0tok + TH],
                                    x_t[2 * dp + i][:, h0tok:h0tok + TH])

        # ================= gates =================
        gexp = persist.tile([8, T], FP32, tag="gexp")
        gate = persist.tile([8, T], FP32R, tag="gate")
        with nc.named_scope("gates"):
            # stage-major over the two token halves: one half's serial
            # softmax chain overlaps the other's matmuls
            pgl = []
            for (n0, nl) in NCH:
                pg = pb(8, TH)
                for k in range(DK):
                    nc.tensor.matmul(pg[:], wgs[:, k, :],
                                     x_t[k][:, n0:n0 + nl],
                                     start=(k == 0), stop=(k == DK - 1))
                nc.scalar.activation(gexp[:, n0:n0 + nl], pg[:], AF.Exp,
                                     bias=bg_col[:])
                pgl.append(pg)
            for ci, (n0, nl) in enumerate(NCH):
                pgs = pa(1, TH)
                nc.tensor.matmul(pgs[:], ones8_col[:], gexp[:, n0:n0 + nl],
                                 start=True, stop=True)
                grec = rows.tile([1, TH], FP32R, tag="grec", bufs=2)
                with nc.allow_low_precision(reason="fp32r matmul operand"):
                    nc.vector.reciprocal(grec[:], pgs[:])
                pgr = pb(8, TH)
                nc.tensor.matmul(pgr[:], ones_row8_r[:], grec[:],
                                 start=True, stop=True)
                nc.vector.tensor_mul(gate[:, n0:n0 + nl],
                                     gexp[:, n0:n0 + nl], pgr[:])

            # moe_acc init = gates^T @ b2   (lhsT = b2 chunks [8, 128])
            for mi in range(DK):
                for (n0, nl) in NCH:
                    pbi = pa(128, TH)
                    nc.tensor.matmul(pbi[:], b2s[:, mi * 128:(mi + 1) * 128],
                                     gate[:, n0:n0 + nl],
                                     start=True, stop=True)
                    nc.scalar.copy(moe[mi][:, n0:n0 + nl], pbi[:])

        # ================= MoE experts (fp8 DoubleRow) =================
        with ExitStack() as ms:
            pmh = ms.enter_context(tc.tile_pool(name="pmh", bufs=26))
            pmw1 = ms.enter_context(tc.tile_pool(name="pmw1", bufs=4))
            pmw2 = ms.enter_context(tc.tile_pool(name="pmw2", bufs=16))
            for e in range(E):
              with nc.named_scope(f"moe{e}"):
                # gate row broadcast to 128 partitions (carries 1/256 descale)
                grep = tmp.tile([128, T], BF16, tag="gerep")
                for (n0, nl) in NCH:
                    pge = pb(128, TH)
                    nc.tensor.matmul(pge[:],
                                     sel8[:, e * 128:(e + 1) * 128],
                                     gate[:, n0:n0 + nl],
                                     start=True, stop=True)
                    nc.vector.tensor_copy(grep[:, n0:n0 + nl], pge[:])

                # expert weights, fp8, DoubleRow pair layout
                w1t = []
                for dp in range(3):
                    wt = pmw1.tile([128, 2, F], F8E4, tag="w1", name="w1t")
                    nc.sync.dma_start(
                        out=wt[:],
                        in_=w1_d[e, dp * 256:(dp + 1) * 256, :].rearrange(
                            "(i p) f -> p i f", p=128))
                    w1t.append(wt)
                w2t = []
                for fbp in range(FK // 2):
                    wt = pmw2.tile([128, 2, D], F8E4, tag="w2", name="w2t")
                    nc.sync.dma_start(
                        out=wt[:],
                        in_=w2_d[e, fbp * 256:(fbp + 1) * 256, :].rearrange(
                            "(i p) d -> p i d", p=128))
                    w2t.append(wt)

                # ---- h = gelu((W1*256)^T x / 256 + b1) -> fp8 [F, T] ----
                hts = []
                for fm in range(FK):
                    fbp, ih = divmod(fm, 2)
                    if ih == 0:
                        hp = pmh.tile([128, 2, T], F8E4, tag="h", name="hp")
                        hts.append(hp)
                    ph0 = pa(128, TH)
                    ph1 = pb(128, TH)
                    for dp in range(3):
                        nc.tensor.matmul(
                            ph0[:], w1t[dp][:, :, fm * 128:(fm + 1) * 128],
                            x8[dp][:, :, 0:TH],
                            start=(dp == 0), stop=(dp == 2), perf_mode=DR)
                        nc.tensor.matmul(
                            ph1[:], w1t[dp][:, :, fm * 128:(fm + 1) * 128],
                            x8[dp][:, :, TH:T],
                            start=(dp == 0), stop=(dp == 2), perf_mode=DR)
                    nc.scalar.activation(hts[fbp][:, ih, 0:TH], ph0[:],
                                         AF.Gelu, bias=b1c[:, e, fm:fm + 1],
                                         scale=1.0 / W8SCALE)
                    nc.scalar.activation(hts[fbp][:, ih, TH:T], ph1[:],
                                         AF.Gelu, bias=b1c[:, e, fm:fm + 1],
                                         scale=1.0 / W8SCALE)

                # ---- y = (W2*256)^T h (K-accum in PSUM), combine ----
                for dg in range(3):
                    pys = [pa(128, TH) for _ in range(2)] + \
                          [pb(128, TH) for _ in range(2)]
                    for fbp in range(FK // 2):
                        for j in range(2):
                            m0 = dg * 256 + j * 128
                            for ni, (n0, nl) in enumerate(NCH):
                                nc.tensor.matmul(
                                    pys[j * 2 + ni][:],
                                    w2t[fbp][:, :, m0:m0 + 128],
                                    hts[fbp][:, :, n0:n0 + nl],
                                    start=(fbp == 0), stop=(fbp == 11),
                                    perf_mode=DR)
                    for j in range(2):
                        mi = dg * 2 + j
                        for ni, (n0, nl) in enumerate(NCH):
                            ty = tmp.tile([128, TH], FP32, tag="ty")
                            nc.vector.tensor_mul(ty[:], pys[j * 2 + ni][:],
                                                 grep[:, n0:n0 + nl])
                            nc.vector.tensor_add(moe[mi][:, n0:n0 + nl],
                                                 moe[mi][:, n0:n0 + nl],
                                                 ty[:])

        # ================= LN2 + output =================
        with ExitStack() as fs:
            pfo = fs.enter_context(tc.tile_pool(name="pfo", bufs=3))
            with nc.named_scope("tail"):
                for (n0, nl) in NCH:
                    for mi in range(DK):
                        nc.vector.tensor_add(x_t[mi][:, n0:n0 + nl],
                                             x_t[mi][:, n0:n0 + nl],
                                             moe[mi][:, n0:n0 + nl])
                layer_norm(
                    [(x_t, l2g_col, l2b_col, moe, n0, n0, nl)
                     for (n0, nl) in NCH])
                for ci in range(2):
                    for (t0, tl) in TTH[ci]:
                        ot = pfo.tile([128, D], FP32, tag="otok")
                        for k in range(DK):
                            ps = pa(128, 128)
                            nc.tensor.transpose(ps[:tl, :],
                                                moe[k][:, t0:t0 + tl],
                                                ident[:, :])
                            nc.vector.tensor_copy(
                                ot[:tl, k * 128:(k + 1) * 128], ps[:tl, :])
                        oeng = nc.sync if (t0 // 128) % 2 == 0 else nc.scalar
                        oeng.dma_start(out=out_d[t0:t0 + tl, :],
                                       in_=ot[:tl, :])

    nc.compile()
    return nc


def _get_nc():
    if "nc" not in _CACHE:
        _CACHE["nc"] = _build()
    return _CACHE["nc"]


def run(inputs, **spmd_kwargs):
    nc = _get_nc()
    f32 = np.float32
    bf16 = ml_dtypes.bfloat16
    f8 = ml_dtypes.float8_e4m3
    inp = {k: np.asarray(v) for k, v in inputs.items()}
    shared = {}
    for name in ("bq", "bk", "bv", "bo", "ln1_g", "ln1_b", "ln2_g", "ln2_b",
                 "Wg", "bg", "b1", "b2"):
        shared[name] = np.ascontiguousarray(inp[name].astype(f32))
    for name in ("Wq", "Wk", "Wv", "Wo"):
        shared[name] = np.ascontiguousarray(inp[name].astype(f32).astype(bf16))
    shared["W1f8"] = np.ascontiguousarray(
        (inp["W1"].astype(f32) * W8SCALE).astype(f8))
    shared["W2f8"] = np.ascontiguousarray(
        (inp["W2"].astype(f32) * W8SCALE).astype(f8))
    sel = np.zeros((E, E * 128), dtype=f32)
    for e in range(E):
        sel[e, e * 128:(e + 1) * 128] = 1.0 / W8SCALE
    shared["sel8"] = sel
    shared["aux_ones"] = np.ones((128, 128), dtype=f32)
    shared["aux_ones_bf"] = np.ones((128, 128), dtype=bf16)
    a64 = np.zeros((65, 128), dtype=f32)
    a64[64, :] = 1.0
    shared["aux_ones64"] = a64
    shared["aux_eps"] = np.full((1, 1), EPS, dtype=f32)
    in_maps = []
    for c in range(NCORES):
        m = dict(shared)
        for name in ("q", "k", "v"):
            pad = np.zeros((TP, D), dtype=bf16)
            pad[:T] = inp[name][c * BPC:(c + 1) * BPC].reshape(
                T, D).astype(f32).astype(bf16)
            # block-major [DK, TP, 128]: each xbar transpose-DMA reads one
            # fully contiguous region
            m[name] = np.ascontiguousarray(
                pad.reshape(TP, DK, 128).transpose(1, 0, 2))
        in_maps.append(m)
    res = run_bass_kernel_spmd(nc, in_maps, core_ids=list(range(NCORES)),
                               **spmd_kwargs)
    out = np.stack([r["out"] for r in res.results])  # [8, T, D]
    return out.reshape(B, S, D), res


def kernel(**inputs):
    out, _ = run(inputs)
    return out
